# revision 1
# baseline (speedup 1.0000x reference)
"""Trainium2 Bass kernel for nn_DataONEEncoder (2-layer GRU + LN + pool + proj + GELU).

Data-parallel over batch: B=256 -> 32 per core on 8 NeuronCores, no collectives.
Per core:
  A: gx0 = xm @ W_ih0^T + b        (fp32r big GEMM, feature-major layouts)
  B: serial T-scan GRU layer 0     (W_hh stationary bf16, h moving bf16 hi+lo)
  C: gx1 = h1 @ W_ih1^T + b        (fp32r big GEMM)
  D: serial T-scan GRU layer 1
  E: LayerNorm + (last + mean-over-T) pooling + proj + exact GELU
"""

import os
import numpy as np
import ml_dtypes

import concourse.bass as bass
from concourse import bacc
import concourse.mybir as mybir
import concourse.tile as tile
from concourse.alu_op_type import AluOpType
from concourse.bass import ts, ds

B, T, F, H = 256, 512, 65, 512
NCORES = 8
BL = B // NCORES          # 32 batch per core
H3 = 3 * H                # 1536
NJ = H3 // 128            # 12 output tiles of the gate dim
NK = H // 128             # 4 contraction tiles of the hidden dim
TB = T * BL               # tokens per core
EPS = 1e-5

f32 = mybir.dt.float32
f32r = mybir.dt.float32r
bf16 = mybir.dt.bfloat16
AF = mybir.ActivationFunctionType

USE_HILO = os.environ.get("KERNEL_HILO", "1") == "1"
SIM_MODE = os.environ.get("KERNEL_SIM", "0") == "1"   # CoreSim lacks Gelu


def r32(ap):
    return ap.bitcast(f32r)


def build_nc(scan_T=T, bl=BL):
    """Build the per-core Bass program. All 8 cores run this same program on
    different batch slices (supplied via in_maps)."""
    tb = scan_T * bl
    nc = bacc.Bacc()

    # ---- external inputs (host pre-laid-out, see kernel()) ----
    xmT = nc.declare_dram_parameter("xmT", [2 * F, tb], f32r, isOutput=False)        # [f, (t,b)]
    w0T = nc.declare_dram_parameter("w0T", [F, 2, H3], f32r, isOutput=False)         # [f, k(x|m), g]
    w1T = nc.declare_dram_parameter("w1T", [128, NK, H3], f32r, isOutput=False)      # [p, k, g]
    whh0 = nc.declare_dram_parameter("whh0", [128, NJ, NK, 128], bf16, isOutput=False)
    whh1 = nc.declare_dram_parameter("whh1", [128, NJ, NK, 128], bf16, isOutput=False)
    gb0 = nc.declare_dram_parameter("gb0", [128, NJ], f32, isOutput=False)          # folded bias
    gb1 = nc.declare_dram_parameter("gb1", [128, NJ], f32, isOutput=False)
    bhn0 = nc.declare_dram_parameter("bhn0", [128, NK], f32, isOutput=False)        # b_hh n-gate
    bhn1 = nc.declare_dram_parameter("bhn1", [128, NK], f32, isOutput=False)
    lng = nc.declare_dram_parameter("lng", [128, NK], f32, isOutput=False)
    lnb = nc.declare_dram_parameter("lnb", [128, NK], f32, isOutput=False)
    wpT = nc.declare_dram_parameter("wpT", [128, NK, 256], f32r, isOutput=False)
    bp = nc.declare_dram_parameter("bp", [128, 2], f32, isOutput=False)
    out = nc.declare_dram_parameter("out", [2, 128, bl], f32, isOutput=True)

    NTOK = min(512, tb)             # tokens per GEMM chunk
    assert tb % NTOK == 0
    nchunks = tb // NTOK
    steps_per_chunk = NTOK // bl    # 16

    with tile.TileContext(nc) as tc:
        with tc.tile_pool(name="dram", bufs=1, space="DRAM") as dram, \
             tc.tile_pool(name="consts", bufs=1) as consts:

            # DRAM intermediates.
            # gx layouts: [t, p, j, b] so each scan step reads one contiguous block.
            gx0T = dram.tile([scan_T, 128, NJ, bl], f32)
            gx1T = dram.tile([scan_T, 128, NJ, bl], f32)
            # h layouts: [k, p, (t,b)] so GEMM-C / phase-E read [128, NTOK] chunks.
            h1T = dram.tile([NK, 128, tb], f32r)
            h2T = dram.tile([NK, 128, tb], f32r)

            # ---- load constants to SBUF ----
            w0_sb = consts.tile([F, 2, H3], f32r)
            nc.sync.dma_start(out=w0_sb, in_=w0T[:])
            w1_sb = consts.tile([128, NK, H3], f32r)
            nc.sync.dma_start(out=w1_sb, in_=w1T[:])
            whh_sb = [consts.tile([128, NJ, NK, 128], bf16, name=f"whh{i}_sb") for i in range(2)]
            nc.sync.dma_start(out=whh_sb[0], in_=whh0[:])
            nc.sync.dma_start(out=whh_sb[1], in_=whh1[:])
            gb_sb = [consts.tile([128, NJ], f32, name=f"gb{i}_sb") for i in range(2)]
            nc.sync.dma_start(out=gb_sb[0], in_=gb0[:])
            nc.sync.dma_start(out=gb_sb[1], in_=gb1[:])
            # broadcast b_hh(n) over batch -> [128, NK, bl]
            bhn_small = [consts.tile([128, NK], f32, name=f"bhn{i}_sm") for i in range(2)]
            bhn_sb = [consts.tile([128, NK, bl], f32, name=f"bhn{i}_sb") for i in range(2)]
            for i, srcp in enumerate((bhn0, bhn1)):
                nc.sync.dma_start(out=bhn_small[i], in_=srcp[:])
                nc.vector.tensor_copy(out=bhn_sb[i],
                                      in_=bhn_small[i].to_broadcast([128, NK, bl]))
            lng_sb = consts.tile([128, NK], f32)
            nc.sync.dma_start(out=lng_sb, in_=lng[:])
            lnb_sb = consts.tile([128, NK], f32)
            nc.sync.dma_start(out=lnb_sb, in_=lnb[:])
            wp_sb = consts.tile([128, NK, 256], f32r)
            nc.sync.dma_start(out=wp_sb, in_=wpT[:])
            bp_sb = consts.tile([128, 2], f32)
            nc.sync.dma_start(out=bp_sb, in_=bp[:])
            ones_stage = consts.tile([128, 128], f32)
            nc.vector.memset(ones_stage, 1.0)
            ones_col = consts.tile([128, 1], f32r)   # lhsT for partition-sum
            nc.vector.tensor_copy(out=ones_col, in_=ones_stage[:, 0:1])
            ones_row = consts.tile([1, 128], f32r)   # lhsT for partition-broadcast
            nc.vector.tensor_copy(out=ones_row, in_=ones_stage[0:1, :])
            eps_sb = consts.tile([1, 1], f32)
            nc.vector.memset(eps_sb, EPS)
            # All consts resident before compute: keeps per-matmul wait
            # counts under the S3_LW sync-wait limit.
            tc.strict_bb_all_engine_barrier()

            # ================= Phase A: gx0 GEMM =================
            with tc.tile_pool(name="a_in", bufs=3) as a_in, \
                 tc.tile_pool(name="a_out", bufs=4) as a_out, \
                 tc.tile_pool(name="a_ps", bufs=4, space="PSUM") as a_ps:
                for c in range(nchunks):
                    tok = ds(c * NTOK, NTOK)
                    xm_sb = a_in.tile([F, 2, NTOK], f32r)
                    nc.sync.dma_start(
                        out=xm_sb,
                        in_=xmT[:, tok].rearrange("(k f) t -> f k t", k=2))
                    for j in range(NJ):
                        ps = a_ps.tile([128, NTOK], f32)
                        nc.tensor.matmul(ps, r32(w0_sb[:, 0, ts(j, 128)]),
                                         r32(xm_sb[:, 0, :]), start=True, stop=False)
                        nc.tensor.matmul(ps, r32(w0_sb[:, 1, ts(j, 128)]),
                                         r32(xm_sb[:, 1, :]), start=False, stop=True)
                        gxs = a_out.tile([128, NTOK], f32)
                        nc.scalar.activation(out=gxs, in_=ps, func=AF.Identity,
                                             bias=gb_sb[0][:, j:j + 1])
                        # write [p, (t,b)] -> gx0T[t, p, j, b]
                        t0 = c * steps_per_chunk
                        dst = gx0T[t0:t0 + steps_per_chunk, :, j, :]
                        nc.sync.dma_start(
                            out=dst.rearrange("t p b -> p t b"),
                            in_=gxs)

            # ================= scan (shared for both layers) =================
            def scan_layer(layer, gxT, houtT):
                whh = whh_sb[layer]
                bhn = bhn_sb[layer]
                with tc.tile_pool(name=f"s{layer}_gx", bufs=6) as gxp, \
                     tc.tile_pool(name=f"s{layer}_h", bufs=3) as hp, \
                     tc.tile_pool(name=f"s{layer}_t", bufs=3) as tp, \
                     tc.tile_pool(name=f"s{layer}_ps", bufs=2, space="PSUM") as psp:
                    h = hp.tile([128, NK, bl], f32, tag="h")
                    nc.vector.memset(h, 0.0)
                    for t in range(scan_T):
                        gx = gxp.tile([128, NJ, bl], f32, tag="gx")
                        nc.sync.dma_start(out=gx, in_=gxT[t])
                        hhi = tp.tile([128, NK, bl], bf16, tag="hhi")
                        nc.vector.tensor_copy(out=hhi, in_=h)
                        if USE_HILO:
                            hlo = tp.tile([128, NK, bl], bf16, tag="hlo")
                            nc.vector.tensor_sub(hlo, h, hhi)
                        ps = psp.tile([128, NJ, bl], f32, tag="ps")
                        for j in range(NJ):
                            for k in range(NK):
                                w = whh[:, j, k, :]
                                nc.tensor.matmul(ps[:, j, :], w, hhi[:, k, :],
                                                 start=(k == 0), stop=(not USE_HILO and k == NK - 1))
                                if USE_HILO:
                                    nc.tensor.matmul(ps[:, j, :], w, hlo[:, k, :],
                                                     start=False, stop=(k == NK - 1))
                        # gates
                        rpre = tp.tile([128, NK, bl], f32, tag="rpre")
                        nc.vector.tensor_add(rpre, ps[:, 0:NK, :], gx[:, 0:NK, :])
                        r = tp.tile([128, NK, bl], f32, tag="r")
                        nc.scalar.activation(out=r, in_=rpre, func=AF.Sigmoid)
                        zpre = tp.tile([128, NK, bl], f32, tag="zpre")
                        nc.vector.tensor_add(zpre, ps[:, NK:2 * NK, :], gx[:, NK:2 * NK, :])
                        z = tp.tile([128, NK, bl], f32, tag="z")
                        nc.scalar.activation(out=z, in_=zpre, func=AF.Sigmoid)
                        nb = tp.tile([128, NK, bl], f32, tag="nb")
                        nc.vector.tensor_add(nb, ps[:, 2 * NK:3 * NK, :], bhn)
                        nh = tp.tile([128, NK, bl], f32, tag="nh")
                        nc.vector.tensor_mul(nh, nb, r)
                        npre = tp.tile([128, NK, bl], f32, tag="npre")
                        nc.vector.tensor_add(npre, nh, gx[:, 2 * NK:3 * NK, :])
                        n = tp.tile([128, NK, bl], f32, tag="n")
                        nc.scalar.activation(out=n, in_=npre, func=AF.Tanh)
                        d = tp.tile([128, NK, bl], f32, tag="d")
                        nc.vector.tensor_sub(d, h, n)
                        zd = tp.tile([128, NK, bl], f32, tag="zd")
                        nc.vector.tensor_mul(zd, z, d)
                        hn = hp.tile([128, NK, bl], f32, tag="h")
                        nc.vector.tensor_add(hn, n, zd)
                        h = hn
                        # h^T out: [p,k,b] -> houtT[k,p,(t,b)]
                        nc.sync.dma_start(
                            out=houtT[:, :, t * bl:(t + 1) * bl].rearrange("k p b -> p k b"),
                            in_=r32(h))

            # ================= Phase B: scan layer 0 =================
            scan_layer(0, gx0T, h1T)

            # ================= Phase C: gx1 GEMM =================
            with tc.tile_pool(name="c_in", bufs=3) as c_in, \
                 tc.tile_pool(name="c_out", bufs=4) as c_out, \
                 tc.tile_pool(name="c_ps", bufs=4, space="PSUM") as c_ps:
                for c in range(nchunks):
                    tok = ds(c * NTOK, NTOK)
                    hch = c_in.tile([128, NK, NTOK], f32r)
                    for k in range(NK):
                        nc.sync.dma_start(out=hch[:, k, :], in_=h1T[k, :, tok])
                    for j in range(NJ):
                        ps = c_ps.tile([128, NTOK], f32)
                        for k in range(NK):
                            nc.tensor.matmul(ps, r32(w1_sb[:, k, ts(j, 128)]),
                                             r32(hch[:, k, :]),
                                             start=(k == 0), stop=(k == NK - 1))
                        gxs = c_out.tile([128, NTOK], f32)
                        nc.scalar.activation(out=gxs, in_=ps, func=AF.Identity,
                                             bias=gb_sb[1][:, j:j + 1])
                        t0 = c * steps_per_chunk
                        nc.sync.dma_start(
                            out=gx1T[t0:t0 + steps_per_chunk, :, j, :].rearrange(
                                "t p b -> p t b"),
                            in_=gxs)

            # ================= Phase D: scan layer 1 =================
            scan_layer(1, gx1T, h2T)

            # ================= Phase E: LN + pool + proj + GELU =================
            with tc.tile_pool(name="e_in", bufs=3) as e_in, \
                 tc.tile_pool(name="e_t", bufs=3) as e_t, \
                 tc.tile_pool(name="e_acc", bufs=1) as e_acc, \
                 tc.tile_pool(name="e_ps", bufs=1, space="PSUM") as e_ps:
                acc = e_acc.tile([128, NK, bl], f32)
                nc.vector.memset(acc, 0.0)
                lastln = e_acc.tile([128, NK, bl], f32)
                for c in range(nchunks):
                    tok = ds(c * NTOK, NTOK)
                    hch = e_in.tile([128, NK, NTOK], f32r, tag="hch")
                    for k in range(NK):
                        nc.sync.dma_start(out=hch[:, k, :], in_=h2T[k, :, tok])
                    sq = e_in.tile([128, NK, NTOK], f32r, tag="sq")
                    nc.scalar.activation(out=sq, in_=hch.bitcast(f32), func=AF.Square)
                    pss = e_ps.tile([1, NTOK], f32, tag="pss")
                    psq = e_ps.tile([1, NTOK], f32, tag="psq")
                    for k in range(NK):
                        nc.tensor.matmul(pss, r32(ones_col), r32(hch[:, k, :]),
                                         start=(k == 0), stop=(k == NK - 1))
                    for k in range(NK):
                        nc.tensor.matmul(psq, r32(ones_col), r32(sq[:, k, :]),
                                         start=(k == 0), stop=(k == NK - 1))
                    mu = e_t.tile([1, NTOK], f32r, tag="mu")
                    nc.vector.tensor_scalar_mul(mu, pss, 1.0 / H)
                    mu2 = e_t.tile([1, NTOK], f32, tag="mu2")
                    nc.vector.tensor_mul(mu2, mu.bitcast(f32), mu.bitcast(f32))
                    var = e_t.tile([1, NTOK], f32, tag="var")
                    nc.vector.scalar_tensor_tensor(var, psq, 1.0 / H, mu2,
                                                   op0=AluOpType.mult,
                                                   op1=AluOpType.subtract)
                    sd = e_t.tile([1, NTOK], f32, tag="sd")
                    nc.scalar.activation(out=sd, in_=var, func=AF.Sqrt, bias=eps_sb)
                    rs = e_t.tile([1, NTOK], f32r, tag="rs")
                    with nc.allow_low_precision(reason="f32r is full-width fp32 bits; rounding happens at the matmul"):
                        nc.vector.reciprocal(rs, sd)
                    bmu = e_ps.tile([128, NTOK], f32, tag="bmu")
                    nc.tensor.matmul(bmu, r32(ones_row), r32(mu))
                    brs = e_ps.tile([128, NTOK], f32, tag="brs")
                    nc.tensor.matmul(brs, r32(ones_row), r32(rs))
                    for k in range(NK):
                        cen = e_t.tile([128, NTOK], f32, tag="cen")
                        nc.vector.tensor_sub(cen, hch[:, k, :].bitcast(f32), bmu)
                        nrm = e_t.tile([128, NTOK], f32, tag="nrm")
                        nc.vector.tensor_mul(nrm, cen, brs)
                        lnv = e_t.tile([128, NTOK], f32, tag="lnv")
                        nc.vector.tensor_scalar(lnv, nrm, lng_sb[:, k:k + 1],
                                                lnb_sb[:, k:k + 1],
                                                op0=AluOpType.mult,
                                                op1=AluOpType.add)
                        red = e_t.tile([128, bl], f32, tag="red")
                        nc.vector.tensor_reduce(
                            red, lnv.rearrange("p (t b) -> p b t", b=bl),
                            axis=mybir.AxisListType.X, op=AluOpType.add)
                        nc.vector.tensor_add(acc[:, k, :], acc[:, k, :], red)
                        if c == nchunks - 1:
                            nc.vector.tensor_copy(
                                out=lastln[:, k, :],
                                in_=lnv[:, (steps_per_chunk - 1) * bl:])
                # pooled = acc/T + ln(h2[T-1])
                po = e_acc.tile([128, NK, bl], f32r)
                nc.vector.scalar_tensor_tensor(po, acc, 1.0 / scan_T, lastln,
                                               op0=AluOpType.mult,
                                               op1=AluOpType.add)
                for j in range(2):
                    psy = e_ps.tile([128, bl], f32, tag="psy")
                    for k in range(NK):
                        nc.tensor.matmul(psy, r32(wp_sb[:, k, ts(j, 128)]),
                                         r32(po[:, k, :]),
                                         start=(k == 0), stop=(k == NK - 1))
                    yj = e_t.tile([128, bl], f32, tag="yj")
                    nc.scalar.activation(out=yj, in_=psy,
                                         func=AF.Identity if SIM_MODE else AF.Gelu,
                                         bias=bp_sb[:, j:j + 1])
                    nc.sync.dma_start(out=out[j], in_=yj)
    nc.finalize()
    return nc


# ---------------- host-side input prep ----------------

def prep_shared(W_ih0, W_hh0, b_ih0, b_hh0, W_ih1, W_hh1, b_ih1, b_hh1,
                ln_g, ln_b, W_proj, b_proj):
    def whh_tiles(W_hh):
        # [p, j, k, m] = W_hh^T[128k+p, 128j+m]
        w = np.ascontiguousarray(W_hh.T).reshape(NK, 128, NJ, 128)
        return np.ascontiguousarray(w.transpose(1, 2, 0, 3)).astype(ml_dtypes.bfloat16)

    def fold_bias(b_ih, b_hh):
        g = b_ih.copy()
        g[:2 * H] += b_hh[:2 * H]
        return np.ascontiguousarray(g.reshape(NJ, 128).T)  # [128, NJ]

    shared = {}
    # w0T[f, k, g] = W_ih0[g, k*F + f]
    w0 = np.ascontiguousarray(W_ih0.T)            # [130, 1536]
    shared["w0T"] = np.ascontiguousarray(w0.reshape(2, F, H3).transpose(1, 0, 2))
    # w1T[p, k, g] = W_ih1[g, 128k+p]
    w1 = np.ascontiguousarray(W_ih1.T)            # [512, 1536]
    shared["w1T"] = np.ascontiguousarray(w1.reshape(NK, 128, H3).transpose(1, 0, 2))
    shared["whh0"] = whh_tiles(W_hh0)
    shared["whh1"] = whh_tiles(W_hh1)
    shared["gb0"] = fold_bias(b_ih0, b_hh0)
    shared["gb1"] = fold_bias(b_ih1, b_hh1)
    shared["bhn0"] = np.ascontiguousarray(b_hh0[2 * H:].reshape(NK, 128).T)
    shared["bhn1"] = np.ascontiguousarray(b_hh1[2 * H:].reshape(NK, 128).T)
    shared["lng"] = np.ascontiguousarray(ln_g.reshape(NK, 128).T)
    shared["lnb"] = np.ascontiguousarray(ln_b.reshape(NK, 128).T)
    # wpT[p, k, c] = W_proj[c, 128k+p]
    shared["wpT"] = np.ascontiguousarray(W_proj.T.reshape(NK, 128, 256).transpose(1, 0, 2))
    shared["bp"] = np.ascontiguousarray(b_proj.reshape(2, 128).T)
    shared = {k: np.asarray(v, dtype=(ml_dtypes.bfloat16 if k.startswith("whh") else np.float32))
              for k, v in shared.items()}
    return shared


def prep_xmT(x_core, mask_core, scan_T=T, bl=BL):
    # xmT[f, t*bl + b] = concat(x, mask)[b, t, f]
    xm = np.concatenate([x_core, mask_core.astype(np.float32)], axis=-1)  # [bl,T,2F]
    return np.ascontiguousarray(xm.transpose(2, 1, 0).reshape(2 * F, scan_T * bl),
                                dtype=np.float32)


_CACHE = {}


def _enable_trace_support():
    """Profiling-only shim (used by test.py, not the graded path): register
    the NTFF profile hook this image's antenv lacks, and keep artifacts
    local instead of uploading."""
    import sys
    import types
    import concourse.bass_utils as bu
    bu.upload_artifacts = lambda tmpdir: "local://" + tmpdir
    try:
        from antenv.axon_hooks import get_axon_ntff_profile_hook  # noqa: F401
        return
    except ImportError:
        pass
    from trn_agent_boot.trn_boot import _ntff_profile_via_ctypes
    hook = _ntff_profile_via_ctypes("/opt/axon/libaxon_pjrt.so")
    mod = types.ModuleType("antenv.axon_hooks")
    mod.get_axon_ntff_profile_hook = lambda: hook
    mod.set_axon_ntff_profile_hook = lambda h: None
    sys.modules["antenv.axon_hooks"] = mod


def kernel(x, mask, W_ih0, W_hh0, b_ih0, b_hh0, W_ih1, W_hh1, b_ih1, b_hh1,
           ln_g, ln_b, W_proj, b_proj):
    from concourse.bass_utils import run_bass_kernel_spmd

    if "nc" not in _CACHE:
        _CACHE["nc"] = build_nc()
    nc = _CACHE["nc"]

    x = np.asarray(x, np.float32)
    mask = np.asarray(mask)
    shared = prep_shared(np.asarray(W_ih0, np.float32), np.asarray(W_hh0, np.float32),
                         np.asarray(b_ih0, np.float32), np.asarray(b_hh0, np.float32),
                         np.asarray(W_ih1, np.float32), np.asarray(W_hh1, np.float32),
                         np.asarray(b_ih1, np.float32), np.asarray(b_hh1, np.float32),
                         np.asarray(ln_g, np.float32), np.asarray(ln_b, np.float32),
                         np.asarray(W_proj, np.float32), np.asarray(b_proj, np.float32))
    in_maps = []
    for c in range(NCORES):
        m = dict(shared)
        m["xmT"] = prep_xmT(x[c * BL:(c + 1) * BL], mask[c * BL:(c + 1) * BL])
        in_maps.append(m)

    trace = os.environ.get("KERNEL_TRACE", "0") == "1"
    kw = {}
    if trace:
        _enable_trace_support()
        kw["tmpdir"] = os.environ.get("KERNEL_TRACE_DIR") or None
    res = run_bass_kernel_spmd(nc, in_maps, list(range(NCORES)), trace=trace, **kw)
    _CACHE["exec_time_ns"] = res.exec_time_ns
    if res.instructions_and_trace is not None:
        _CACHE["trace_path"] = res.instructions_and_trace[1]
    outs = []
    for c in range(NCORES):
        y = res.results[c]["out"]          # [2, 128, BL]
        outs.append(y.reshape(256, BL).T)  # [BL, 256]
    return np.ascontiguousarray(np.concatenate(outs, axis=0), dtype=np.float32)



# revision 14
# speedup vs baseline: 1.8320x; 1.8320x over previous
"""Trainium2 Bass kernel for nn_DataONEEncoder (2-layer GRU + LN + pool + proj + GELU).

Fully-fused pipeline, data-parallel over batch (B=256 -> 32 per core, 8 cores).

All intermediates stay in SBUF (no DRAM round trips for gx/h):
  - A-GEMM  : gx0 = xm @ W_ih0^T + b   computed chunk-by-chunk (16 steps) into an
              SBUF ring, interleaved into the scan as PE filler work.
  - L0 scan : GRU layer 0, one step per iteration.
  - C-GEMM  : gx1 = h1 @ W_ih1^T + b   from the L0 h-ring, PE filler work,
              one chunk behind L0.
  - L1 scan : GRU layer 1, two chunks behind L0 (so C can spread out).
  - E       : LayerNorm stats + pooling accumulation per chunk, three chunks
              behind L0;  mean_t LN(h2) = g*(sum_t h2*rs - sum_t mu*rs)/T + b.

Per scan step the gate-input adds (gx_r, gx_z) and b_hh(n) are folded into the
PSUM accumulation with identity / rank-1 matmuls, so the vector chain is only
5 tensor_tensor ops:  t1 = z*h ; nh = r*ps_n ; npre = nh+gx_n ; t2 = (1-z)*n ;
h' = t1+t2  (with r, z, 1-z, tanh on the scalar engine).  h is stored bf16.

The two layers' scans interleave at step granularity: while the PE runs one
layer's matmul burst, the DVE/ACT run the other layer's gate chain, keeping
the PE warm (HAM) and all engines busy.
"""

import os
import numpy as np
import ml_dtypes

import concourse.bass as bass
from concourse import bacc
import concourse.mybir as mybir
import concourse.tile as tile
from concourse.alu_op_type import AluOpType
from concourse.bass import ts, ds

B, T, F, H = 256, 512, 65, 512
NCORES = 8
BL = B // NCORES          # 32 batch per core
H3 = 3 * H                # 1536
NJ = H3 // 128            # 12 gate j-tiles
NK = H // 128             # 4 hidden k-tiles
EPS = 1e-5
CH = 16                   # scan steps per chunk

f32 = mybir.dt.float32
f32r = mybir.dt.float32r
bf16 = mybir.dt.bfloat16
AF = mybir.ActivationFunctionType
AX = mybir.AxisListType

SIM_MODE = os.environ.get("KERNEL_SIM", "0") == "1"   # CoreSim lacks Gelu
ID_FOLD = os.environ.get("KERNEL_IDFOLD", "1") == "1"


def r32(ap):
    return ap.bitcast(f32r)


def build_nc(scan_T=T, bl=BL):
    assert scan_T % CH == 0
    nch = scan_T // CH
    ctok = CH * bl
    nc = bacc.Bacc()

    # ---- external inputs (host pre-laid-out, see kernel()) ----
    xmT = nc.declare_dram_parameter("xmT", [2 * F, scan_T * bl], f32r, isOutput=False)
    w0T = nc.declare_dram_parameter("w0T", [F, 2, H3], f32r, isOutput=False)
    w1T = nc.declare_dram_parameter("w1T", [128, NK, H3], bf16, isOutput=False)
    whh0 = nc.declare_dram_parameter("whh0", [128, NJ, NK, 128], bf16, isOutput=False)
    whh1 = nc.declare_dram_parameter("whh1", [128, NJ, NK, 128], bf16, isOutput=False)
    gb0 = nc.declare_dram_parameter("gb0", [128, NJ], f32, isOutput=False)
    gb1 = nc.declare_dram_parameter("gb1", [128, NJ], f32, isOutput=False)
    bhnT = nc.declare_dram_parameter("bhnT", [1, 2, NK, 128], bf16, isOutput=False)
    ident = nc.declare_dram_parameter("ident", [128, 128], f32, isOutput=False)
    lng = nc.declare_dram_parameter("lng", [128, NK], f32, isOutput=False)
    lnb2 = nc.declare_dram_parameter("lnb2", [128, NK], f32, isOutput=False)
    wpT = nc.declare_dram_parameter("wpT", [128, NK, 256], f32r, isOutput=False)
    bp = nc.declare_dram_parameter("bp", [128, 2], f32, isOutput=False)
    out = nc.declare_dram_parameter("out", [2, 128, bl], f32, isOutput=True)

    with tile.TileContext(nc) as tc:
        with tc.tile_pool(name="consts", bufs=1) as consts:

            # ---- constants to SBUF ----
            w0_sb = consts.tile([F, 2, H3], f32r)
            nc.sync.dma_start(out=w0_sb, in_=w0T[:])
            w1_sb = consts.tile([128, NK, H3], bf16)
            nc.sync.dma_start(out=w1_sb, in_=w1T[:])
            whh_sb = [consts.tile([128, NJ, NK, 128], bf16, name=f"whh{i}_sb")
                      for i in range(2)]
            nc.sync.dma_start(out=whh_sb[0], in_=whh0[:])
            nc.sync.dma_start(out=whh_sb[1], in_=whh1[:])
            gb_sb = [consts.tile([128, NJ], f32, name=f"gb{i}_sb") for i in range(2)]
            nc.sync.dma_start(out=gb_sb[0], in_=gb0[:])
            nc.sync.dma_start(out=gb_sb[1], in_=gb1[:])
            bhn_sb = consts.tile([1, 2, NK, 128], bf16)
            nc.sync.dma_start(out=bhn_sb, in_=bhnT[:])
            id_sb = consts.tile([128, 128], f32r)
            nc.sync.dma_start(out=id_sb, in_=ident[:].bitcast(f32r))
            id_sbb = consts.tile([128, 128], bf16)
            nc.vector.tensor_copy(out=id_sbb, in_=id_sb.bitcast(f32))
            lng_sb = consts.tile([128, NK], f32)
            nc.sync.dma_start(out=lng_sb, in_=lng[:])
            lnb2_sb = consts.tile([128, NK], f32)
            nc.sync.dma_start(out=lnb2_sb, in_=lnb2[:])
            wp_sb = consts.tile([128, NK, 256], f32r)
            nc.sync.dma_start(out=wp_sb, in_=wpT[:])
            bp_sb = consts.tile([128, 2], f32)
            nc.sync.dma_start(out=bp_sb, in_=bp[:])

            ones_stage = consts.tile([128, 128], f32)
            nc.vector.memset(ones_stage, 1.0)
            ones_col = consts.tile([128, 1], f32r)     # lhsT for partition-sum
            nc.vector.tensor_copy(out=ones_col, in_=ones_stage[:, 0:1])
            ones_row = consts.tile([1, 128], f32r)     # lhsT for partition-bcast
            nc.vector.tensor_copy(out=ones_row, in_=ones_stage[0:1, :])
            ones_blb = consts.tile([1, bl], bf16)      # rhs for rank-1 bias mm
            nc.vector.tensor_copy(out=ones_blb, in_=ones_stage[0:1, 0:bl])
            ones_colb = consts.tile([128, 1], bf16)    # lhsT for bf16 partition-sum
            nc.vector.tensor_copy(out=ones_colb, in_=ones_stage[:, 0:1])
            eps_sb = consts.tile([1, 1], f32)
            nc.vector.memset(eps_sb, EPS)

            hz = consts.tile([128, NK, bl], bf16)      # h(0) = 0
            nc.vector.memset(hz, 0.0)

            # E accumulators
            s1_acc = consts.tile([128, NK, bl], f32)   # sum_t h2*rs
            nc.vector.memset(s1_acc, 0.0)
            s2_acc = consts.tile([1, bl], f32)         # sum_t mu*rs
            nc.vector.memset(s2_acc, 0.0)
            rs_last = consts.tile([1, bl], f32r)
            mu_last = consts.tile([1, bl], f32r)

            tc.strict_bb_all_engine_barrier()

            with tc.tile_pool(name="gx0", bufs=2) as gx0p, \
                 tc.tile_pool(name="gx1", bufs=2) as gx1p, \
                 tc.tile_pool(name="h1", bufs=2) as h1p, \
                 tc.tile_pool(name="h2", bufs=2) as h2p, \
                 tc.tile_pool(name="xm", bufs=2) as xmp, \
                 tc.tile_pool(name="tmp", bufs=2) as tmp, \
                 tc.tile_pool(name="et", bufs=1) as etp, \
                 tc.tile_pool(name="ep", bufs=1) as ep:

                h2_keep = ep.tile([128, NK, bl], f32)  # h2(T) copy for epilogue

                with tc.tile_pool(name="ps0", bufs=2, space="PSUM") as ps0p, \
                     tc.tile_pool(name="ps1", bufs=2, space="PSUM") as ps1p, \
                     tc.tile_pool(name="psA", bufs=1, space="PSUM") as psAp, \
                     tc.tile_pool(name="psC", bufs=1, space="PSUM") as psCp, \
                     tc.tile_pool(name="psE", bufs=1, space="PSUM") as psEp:

                    # ---------------- emission helpers ----------------
                    ps_pool = [ps0p, ps1p]

                    def emit_scan_mms(l, gx_slot, i, h_prev):
                        """One scan step's matmul burst for layer l.
                        ps[:,j,:] = W_hh[j]@h (+ gx for r,z; + b_hh_n for n).
                        h_prev = (tile, islice or None)."""
                        ps = ps_pool[l].tile([128, NJ, bl], f32, tag=f"ps{l}")
                        tsl = ts(i, bl)
                        ht, hi = h_prev
                        def hk(k):
                            return ht[:, k, :] if hi is None \
                                else ht[:, k, ts(hi, bl)]
                        # n-gate groups first (j = 2NK..3NK): rank-1 bias, then W
                        for q in range(NK):
                            j = 2 * NK + q
                            nc.tensor.matmul(ps[:, j, :], bhn_sb[:, l, q, :],
                                             ones_blb, start=True, stop=False)
                            for k in range(NK):
                                nc.tensor.matmul(ps[:, j, :], whh_sb[l][:, j, k, :],
                                                 hk(k),
                                                 start=False, stop=(k == NK - 1))
                        # r,z groups: gx identity-fold, then W
                        for j in range(2 * NK):
                            if ID_FOLD:
                                if l == 0:
                                    nc.tensor.matmul(ps[:, j, :], id_sb,
                                                     gx_slot[:, j, tsl],
                                                     start=True, stop=False)
                                else:
                                    nc.tensor.matmul(ps[:, j, :], id_sbb,
                                                     gx_slot[:, j, tsl],
                                                     start=True, stop=False)
                            for k in range(NK):
                                nc.tensor.matmul(ps[:, j, :], whh_sb[l][:, j, k, :],
                                                 hk(k),
                                                 start=(k == 0 and not ID_FOLD),
                                                 stop=(k == NK - 1))
                        return ps

                    def emit_chain(l, ps, gx_slot, i, h_prev, h_out):
                        """Gate math for one step; writes bf16 h' into h_out."""
                        tsl = ts(i, bl)
                        ht, hi = h_prev
                        h_read = ht if hi is None else ht[:, :, ts(hi, bl)]
                        if l == 0:
                            gx_slot = gx_slot.bitcast(f32)
                        if ID_FOLD:
                            rz = tmp.tile([128, 2 * NK, bl], f32, tag=f"rz{l}")
                            nc.scalar.activation(out=rz, in_=ps[:, 0:2 * NK, :],
                                                 func=AF.Sigmoid)
                            u = tmp.tile([128, NK, bl], f32, tag=f"u{l}")
                            nc.scalar.activation(out=u, in_=ps[:, NK:2 * NK, :],
                                                 func=AF.Sigmoid, scale=-1.0)
                        else:
                            rzp = tmp.tile([128, 2 * NK, bl], f32, tag=f"rzp{l}")
                            nc.vector.tensor_add(rzp, ps[:, 0:2 * NK, :],
                                                 gx_slot[:, 0:2 * NK, tsl])
                            rz = tmp.tile([128, 2 * NK, bl], f32, tag=f"rz{l}")
                            nc.scalar.activation(out=rz, in_=rzp, func=AF.Sigmoid)
                            u = tmp.tile([128, NK, bl], f32, tag=f"u{l}")
                            nc.scalar.activation(out=u, in_=rzp[:, NK:2 * NK, :],
                                                 func=AF.Sigmoid, scale=-1.0)
                        t1 = tmp.tile([128, NK, bl], f32, tag=f"t1{l}")
                        nc.vector.tensor_mul(t1, rz[:, NK:2 * NK, :], h_read)
                        nh = tmp.tile([128, NK, bl], f32, tag=f"nh{l}")
                        nc.vector.tensor_mul(nh, ps[:, 2 * NK:3 * NK, :],
                                             rz[:, 0:NK, :])
                        npre = tmp.tile([128, NK, bl], f32, tag=f"np{l}")
                        nc.vector.tensor_add(npre, nh,
                                             gx_slot[:, 2 * NK:3 * NK, tsl])
                        n = tmp.tile([128, NK, bl], f32, tag=f"n{l}")
                        nc.scalar.activation(out=n, in_=npre, func=AF.Tanh)
                        t2 = tmp.tile([128, NK, bl], f32, tag=f"t2{l}")
                        nc.vector.tensor_mul(t2, u, n)
                        nc.vector.tensor_add(h_out, t1, t2)

                    def emit_A_unit(j, xs, gx_slot):
                        ps = psAp.tile([128, ctok], f32, tag="A")
                        nc.tensor.matmul(ps, r32(w0_sb[:, 0, ts(j, 128)]),
                                         r32(xs[:, 0, :]), start=True, stop=False)
                        nc.tensor.matmul(ps, r32(w0_sb[:, 1, ts(j, 128)]),
                                         r32(xs[:, 1, :]), start=False, stop=True)
                        nc.scalar.activation(out=gx_slot[:, j, :], in_=ps,
                                             func=AF.Identity,
                                             bias=gb_sb[0][:, j:j + 1])

                    def emit_C_unit(j, h1_slot, gx_slot):
                        ps = psCp.tile([128, ctok], f32, tag="C")
                        for k in range(NK):
                            nc.tensor.matmul(ps, w1_sb[:, k, ts(j, 128)],
                                             h1_slot[:, k, :],
                                             start=(k == 0), stop=(k == NK - 1))
                        nc.vector.tensor_scalar_add(gx_slot[:, j, :], ps,
                                                    gb_sb[1][:, j:j + 1])

                    def emit_E(h2_slot, is_last):
                        """LN stats + pooling accumulation over one chunk."""
                        sq = etp.tile([128, NK, ctok], bf16, tag="sq")
                        nc.scalar.activation(out=sq, in_=h2_slot, func=AF.Square)
                        pss = psEp.tile([1, ctok], f32, tag="ps")
                        for k in range(NK):
                            nc.tensor.matmul(pss, ones_colb, h2_slot[:, k, :],
                                             start=(k == 0), stop=(k == NK - 1))
                        # packed per-token scratch: 0=mu 1=mu2/sd/mrs 2=var
                        esc = etp.tile([1, 4, ctok], f32, tag="esc")
                        mu = esc[:, 0, :]
                        nc.vector.tensor_scalar_mul(mu, pss, 1.0 / H)
                        psq = psEp.tile([1, ctok], f32, tag="ps")
                        for k in range(NK):
                            nc.tensor.matmul(psq, ones_colb, sq[:, k, :],
                                             start=(k == 0), stop=(k == NK - 1))
                        mu2 = esc[:, 1, :]
                        nc.vector.tensor_mul(mu2, mu, mu)
                        var = esc[:, 2, :]
                        nc.vector.scalar_tensor_tensor(var, psq, 1.0 / H, mu2,
                                                       op0=AluOpType.mult,
                                                       op1=AluOpType.subtract)
                        sd = esc[:, 1, :]
                        nc.scalar.activation(out=sd, in_=var, func=AF.Sqrt,
                                             bias=eps_sb)
                        rs = etp.tile([1, ctok], f32r, tag="rs")
                        with nc.allow_low_precision(reason="f32r is full-width fp32"):
                            nc.vector.reciprocal(rs, sd)
                        brs = psEp.tile([128, ctok], f32, tag="brs")
                        nc.tensor.matmul(brs, ones_row, rs)
                        for k in range(NK):
                            t = etp.tile([128, ctok], f32, tag="et")
                            nc.vector.tensor_mul(t, h2_slot[:, k, :], brs)
                            red = etp.tile([128, bl], f32, tag="red")
                            nc.vector.tensor_reduce(
                                red, t.rearrange("p (t b) -> p b t", b=bl),
                                axis=AX.X, op=AluOpType.add)
                            nc.vector.tensor_add(s1_acc[:, k, :], s1_acc[:, k, :],
                                                 red)
                        mrs = esc[:, 1, :]
                        nc.vector.tensor_mul(mrs, mu, rs.bitcast(f32))
                        redm = etp.tile([1, bl], f32, tag="redm")
                        nc.vector.tensor_reduce(
                            redm, mrs.rearrange("p (t b) -> p b t", b=bl),
                            axis=AX.X, op=AluOpType.add)
                        nc.vector.tensor_add(s2_acc, s2_acc, redm)
                        if is_last:
                            nc.vector.tensor_copy(out=rs_last,
                                                  in_=rs[:, (CH - 1) * bl:].bitcast(f32))
                            nc.vector.tensor_copy(out=mu_last,
                                                  in_=mu[:, (CH - 1) * bl:])
                            nc.vector.tensor_copy(
                                out=h2_keep, in_=h2_slot[:, :, (CH - 1) * bl:])

                    # ---------------- main pipeline ----------------
                    # iteration c: A(c+1) fillers, L0 chunk c, C(c-1) fillers,
                    #              L1 chunk c-2, E(c-3)
                    L1_LAG = 2
                    xm_tiles = {}
                    gx0_slot = {}
                    gx1_slot = {}
                    h1_slot = {}
                    h2_slot = {}
                    h1_prev = (hz, None)  # h-state entering next L0 step
                    h2_prev = (hz, None)

                    def load_xm(c):
                        xs = xmp.tile([F, 2, ctok], f32r, tag="xm", name="xms")
                        nc.sync.dma_start(
                            out=xs,
                            in_=xmT[:, ds(c * ctok, ctok)].rearrange(
                                "(k f) t -> f k t", k=2))
                        xm_tiles[c] = xs

                    # prologue: A(0) fully, so L0 can start immediately
                    load_xm(0)
                    gx0_slot[0] = gx0p.tile([128, NJ, ctok], f32r, tag="gx0", name="gx0s")
                    for j in range(NJ):
                        emit_A_unit(j, xm_tiles[0], gx0_slot[0])
                    del xm_tiles[0]
                    load_xm(1)

                    for c in range(nch + L1_LAG + 1):
                        a_c = c + 1          # A chunk this iteration
                        l0_c = c             # L0 chunk
                        c_c = c - 1          # C chunk
                        l1_c = c - L1_LAG    # L1 chunk
                        e_c = c - L1_LAG - 1 # E chunk

                        if a_c < nch:
                            gx0_slot[a_c] = gx0p.tile([128, NJ, ctok], f32r,
                                                      tag="gx0", name="gx0s")
                            if a_c + 1 < nch:
                                load_xm(a_c + 1)
                        if l0_c < nch:
                            h1_slot[l0_c] = h1p.tile([128, NK, ctok], bf16,
                                                     tag="h1", name="h1s")
                        if 0 <= c_c < nch:
                            gx1_slot[c_c] = gx1p.tile([128, NJ, ctok], bf16,
                                                      tag="gx1", name="gx1s")
                        if 0 <= l1_c < nch:
                            h2_slot[l1_c] = h2p.tile([128, NK, ctok], bf16,
                                                     tag="h2", name="h2s")

                        for i in range(CH):
                            if l0_c < nch:
                                ps0 = emit_scan_mms(0, gx0_slot[l0_c], i, h1_prev)
                            if 0 <= l1_c < nch:
                                ps1 = emit_scan_mms(1, gx1_slot[l1_c], i, h2_prev)
                            if l0_c < nch:
                                h_out = h1_slot[l0_c][:, :, ts(i, bl)]
                                emit_chain(0, ps0, gx0_slot[l0_c], i, h1_prev,
                                           h_out)
                                h1_prev = (h1_slot[l0_c], i)
                            if 0 <= l1_c < nch:
                                h_out = h2_slot[l1_c][:, :, ts(i, bl)]
                                emit_chain(1, ps1, gx1_slot[l1_c], i, h2_prev,
                                           h_out)
                                h2_prev = (h2_slot[l1_c], i)
                            # PE fillers
                            if i < NJ:
                                if a_c < nch:
                                    emit_A_unit(i, xm_tiles[a_c], gx0_slot[a_c])
                                if 0 <= c_c < nch:
                                    emit_C_unit(i, h1_slot[c_c], gx1_slot[c_c])

                        if a_c < nch:
                            del xm_tiles[a_c]
                        if 0 <= e_c < nch:
                            emit_E(h2_slot[e_c], is_last=(e_c == nch - 1))

                # ------------ epilogue: pool + proj + GELU ------------
                with tc.tile_pool(name="psF", bufs=1, space="PSUM") as psFp:
                    # broadcasts of per-token scalars to 128 partitions
                    bc = psFp.tile([128, 3, bl], f32, tag="bc")
                    s2t = ep.tile([1, bl], f32r)
                    nc.vector.tensor_scalar_mul(s2t, s2_acc, 1.0 / scan_T)
                    nc.tensor.matmul(bc[:, 0, :], ones_row, s2t)
                    nc.tensor.matmul(bc[:, 1, :], ones_row, mu_last)
                    nc.tensor.matmul(bc[:, 2, :], ones_row, rs_last)
                    # mean part: pm = S1/T - bcast(s2/T)
                    pm = ep.tile([128, NK, bl], f32)
                    nc.vector.scalar_tensor_tensor(
                        pm, s1_acc, 1.0 / scan_T,
                        bc[:, 0:1, :].to_broadcast([128, NK, bl]),
                        op0=AluOpType.mult, op1=AluOpType.subtract)
                    # last part: (h2_last - mu)*rs
                    hl = ep.tile([128, NK, bl], f32)
                    nc.vector.tensor_sub(
                        hl, h2_keep, bc[:, 1:2, :].to_broadcast([128, NK, bl]))
                    hlr = ep.tile([128, NK, bl], f32)
                    nc.vector.tensor_mul(
                        hlr, hl, bc[:, 2:3, :].to_broadcast([128, NK, bl]))
                    both = ep.tile([128, NK, bl], f32)
                    nc.vector.tensor_add(both, pm, hlr)
                    # pooled = g*both + 2*b  (LN affine applied to both terms)
                    po = ep.tile([128, NK, bl], f32r)
                    for k in range(NK):
                        nc.vector.tensor_scalar(po[:, k, :], both[:, k, :],
                                                lng_sb[:, k:k + 1],
                                                lnb2_sb[:, k:k + 1],
                                                op0=AluOpType.mult,
                                                op1=AluOpType.add)
                    for j in range(2):
                        psy = psFp.tile([128, bl], f32, tag="psy")
                        for k in range(NK):
                            nc.tensor.matmul(psy, r32(wp_sb[:, k, ts(j, 128)]),
                                             po[:, k, :],
                                             start=(k == 0), stop=(k == NK - 1))
                        yj = ep.tile([128, bl], f32, name=f"yj{j}")
                        nc.scalar.activation(
                            out=yj, in_=psy,
                            func=AF.Identity if SIM_MODE else AF.Gelu,
                            bias=bp_sb[:, j:j + 1])
                        nc.sync.dma_start(out=out[j], in_=yj)
    nc.finalize()
    return nc


# ---------------- host-side input prep ----------------

def prep_shared(W_ih0, W_hh0, b_ih0, b_hh0, W_ih1, W_hh1, b_ih1, b_hh1,
                ln_g, ln_b, W_proj, b_proj):
    def whh_tiles(W_hh):
        # [p, j, k, m] = W_hh^T[128k+p, 128j+m]
        w = np.ascontiguousarray(W_hh.T).reshape(NK, 128, NJ, 128)
        return np.ascontiguousarray(w.transpose(1, 2, 0, 3)).astype(ml_dtypes.bfloat16)

    def fold_bias(b_ih, b_hh):
        g = b_ih.copy()
        g[:2 * H] += b_hh[:2 * H]
        return np.ascontiguousarray(g.reshape(NJ, 128).T)  # [128, NJ]

    shared = {}
    w0 = np.ascontiguousarray(W_ih0.T)            # [130, 1536]
    shared["w0T"] = np.ascontiguousarray(w0.reshape(2, F, H3).transpose(1, 0, 2))
    w1 = np.ascontiguousarray(W_ih1.T)            # [512, 1536]
    shared["w1T"] = np.ascontiguousarray(w1.reshape(NK, 128, H3).transpose(1, 0, 2)).astype(ml_dtypes.bfloat16)
    shared["whh0"] = whh_tiles(W_hh0)
    shared["whh1"] = whh_tiles(W_hh1)
    shared["gb0"] = fold_bias(b_ih0, b_hh0)
    shared["gb1"] = fold_bias(b_ih1, b_hh1)
    bhn = np.stack([b_hh0[2 * H:].reshape(NK, 128),
                    b_hh1[2 * H:].reshape(NK, 128)])   # [2, NK, 128]
    shared["bhnT"] = bhn[None]
    shared["ident"] = np.eye(128, dtype=np.float32)
    shared["lng"] = np.ascontiguousarray(ln_g.reshape(NK, 128).T)
    shared["lnb2"] = np.ascontiguousarray((2.0 * ln_b).reshape(NK, 128).T)
    shared["wpT"] = np.ascontiguousarray(
        W_proj.T.reshape(NK, 128, 256).transpose(1, 0, 2))
    shared["bp"] = np.ascontiguousarray(b_proj.reshape(2, 128).T)
    shared = {k: np.asarray(v, dtype=(ml_dtypes.bfloat16
                                      if k in ("whh0", "whh1", "bhnT", "w1T")
                                      else np.float32))
              for k, v in shared.items()}
    return shared


def prep_xmT(x_core, mask_core, scan_T=T, bl=BL):
    # xmT[f, t*bl + b] = concat(x, mask)[b, t, f]
    xm = np.concatenate([x_core, mask_core.astype(np.float32)], axis=-1)  # [bl,T,2F]
    return np.ascontiguousarray(xm.transpose(2, 1, 0).reshape(2 * F, scan_T * bl),
                                dtype=np.float32)


_CACHE = {}


def _enable_trace_support():
    """Profiling-only shim (used by test.py, not the graded path)."""
    import sys
    import types
    import concourse.bass_utils as bu
    bu.upload_artifacts = lambda tmpdir: "local://" + tmpdir
    try:
        from antenv.axon_hooks import get_axon_ntff_profile_hook  # noqa: F401
        return
    except ImportError:
        pass
    from trn_agent_boot.trn_boot import _ntff_profile_via_ctypes
    hook = _ntff_profile_via_ctypes("/opt/axon/libaxon_pjrt.so")
    mod = types.ModuleType("antenv.axon_hooks")
    mod.get_axon_ntff_profile_hook = lambda: hook
    mod.set_axon_ntff_profile_hook = lambda h: None
    sys.modules["antenv.axon_hooks"] = mod


def kernel(x, mask, W_ih0, W_hh0, b_ih0, b_hh0, W_ih1, W_hh1, b_ih1, b_hh1,
           ln_g, ln_b, W_proj, b_proj):
    from concourse.bass_utils import run_bass_kernel_spmd

    if "nc" not in _CACHE:
        _CACHE["nc"] = build_nc()
    nc = _CACHE["nc"]

    x = np.asarray(x, np.float32)
    mask = np.asarray(mask)
    shared = prep_shared(np.asarray(W_ih0, np.float32), np.asarray(W_hh0, np.float32),
                         np.asarray(b_ih0, np.float32), np.asarray(b_hh0, np.float32),
                         np.asarray(W_ih1, np.float32), np.asarray(W_hh1, np.float32),
                         np.asarray(b_ih1, np.float32), np.asarray(b_hh1, np.float32),
                         np.asarray(ln_g, np.float32), np.asarray(ln_b, np.float32),
                         np.asarray(W_proj, np.float32), np.asarray(b_proj, np.float32))
    in_maps = []
    for c in range(NCORES):
        m = dict(shared)
        m["xmT"] = prep_xmT(x[c * BL:(c + 1) * BL], mask[c * BL:(c + 1) * BL])
        in_maps.append(m)

    trace = os.environ.get("KERNEL_TRACE", "0") == "1"
    kw = {}
    if trace:
        _enable_trace_support()
        kw["tmpdir"] = os.environ.get("KERNEL_TRACE_DIR") or None
    res = run_bass_kernel_spmd(nc, in_maps, list(range(NCORES)), trace=trace, **kw)
    _CACHE["exec_time_ns"] = res.exec_time_ns
    if res.instructions_and_trace is not None:
        _CACHE["trace_path"] = res.instructions_and_trace[1]
    outs = []
    for c in range(NCORES):
        y = res.results[c]["out"]          # [2, 128, BL]
        outs.append(y.reshape(256, BL).T)  # [BL, 256]
    return np.ascontiguousarray(np.concatenate(outs, axis=0), dtype=np.float32)


# revision 15
# speedup vs baseline: 2.5175x; 1.3742x over previous
"""Trainium2 Bass kernel for nn_DataONEEncoder (2-layer GRU + LN + pool + proj + GELU).

Fully-fused pipeline, data-parallel over batch (B=256 -> 32 per core, 8 cores).

All intermediates stay in SBUF (no DRAM round trips for gx/h):
  - A-GEMM  : gx0 = xm @ W_ih0^T + b   computed chunk-by-chunk (16 steps) into an
              SBUF ring, interleaved into the scan as PE filler work.
  - L0 scan : GRU layer 0, one step per iteration.
  - C-GEMM  : gx1 = h1 @ W_ih1^T + b   from the L0 h-ring, PE filler work,
              one chunk behind L0.
  - L1 scan : GRU layer 1, two chunks behind L0 (so C can spread out).
  - E       : LayerNorm stats + pooling accumulation per chunk, three chunks
              behind L0;  mean_t LN(h2) = g*(sum_t h2*rs - sum_t mu*rs)/T + b.

Per scan step the gate-input adds (gx_r, gx_z) and b_hh(n) are folded into the
PSUM accumulation with identity / rank-1 matmuls, so the vector chain is only
5 tensor_tensor ops:  t1 = z*h ; nh = r*ps_n ; npre = nh+gx_n ; t2 = (1-z)*n ;
h' = t1+t2  (with r, z, 1-z, tanh on the scalar engine).  h is stored bf16.

The two layers' scans interleave at step granularity: while the PE runs one
layer's matmul burst, the DVE/ACT run the other layer's gate chain, keeping
the PE warm (HAM) and all engines busy.
"""

import os
import numpy as np
import ml_dtypes

import concourse.bass as bass
from concourse import bacc
import concourse.mybir as mybir
import concourse.tile as tile
from concourse.alu_op_type import AluOpType
from concourse.bass import ts, ds

B, T, F, H = 256, 512, 65, 512
NCORES = 8
BL = B // NCORES          # 32 batch per core
H3 = 3 * H                # 1536
NJ = H3 // 128            # 12 gate j-tiles
NK = H // 128             # 4 hidden k-tiles
EPS = 1e-5
CH = 16                   # scan steps per chunk

f32 = mybir.dt.float32
f32r = mybir.dt.float32r
bf16 = mybir.dt.bfloat16
AF = mybir.ActivationFunctionType
AX = mybir.AxisListType

SIM_MODE = os.environ.get("KERNEL_SIM", "0") == "1"   # CoreSim lacks Gelu
ID_FOLD = os.environ.get("KERNEL_IDFOLD", "1") == "1"


def r32(ap):
    return ap.bitcast(f32r)


def build_nc(scan_T=T, bl=BL):
    assert scan_T % CH == 0
    nch = scan_T // CH
    ctok = CH * bl
    nc = bacc.Bacc()

    # ---- external inputs (host pre-laid-out, see kernel()) ----
    xmT = nc.declare_dram_parameter("xmT", [2 * F, scan_T * bl], bf16, isOutput=False)
    w0T = nc.declare_dram_parameter("w0T", [F, 2, H3], bf16, isOutput=False)
    w1T = nc.declare_dram_parameter("w1T", [128, NK, H3], bf16, isOutput=False)
    whh0 = nc.declare_dram_parameter("whh0", [128, NJ, NK, 128], bf16, isOutput=False)
    whh1 = nc.declare_dram_parameter("whh1", [128, NJ, NK, 128], bf16, isOutput=False)
    gb0 = nc.declare_dram_parameter("gb0", [128, NJ], f32, isOutput=False)
    gb1 = nc.declare_dram_parameter("gb1", [128, NJ], f32, isOutput=False)
    bhnT = nc.declare_dram_parameter("bhnT", [128, 2, NK], f32, isOutput=False)
    ident = nc.declare_dram_parameter("ident", [128, 128], bf16, isOutput=False)
    lng = nc.declare_dram_parameter("lng", [128, NK], f32, isOutput=False)
    lnb2 = nc.declare_dram_parameter("lnb2", [128, NK], f32, isOutput=False)
    wpT = nc.declare_dram_parameter("wpT", [128, NK, 256], f32r, isOutput=False)
    bp = nc.declare_dram_parameter("bp", [128, 2], f32, isOutput=False)
    out = nc.declare_dram_parameter("out", [2, 128, bl], f32, isOutput=True)

    with tile.TileContext(nc) as tc:
        with tc.tile_pool(name="consts", bufs=1) as consts:

            # ---- constants to SBUF ----
            w0_sb = consts.tile([F, 2, H3], bf16)
            nc.sync.dma_start(out=w0_sb, in_=w0T[:])
            w1_sb = consts.tile([128, NK, H3], bf16)
            nc.sync.dma_start(out=w1_sb, in_=w1T[:])
            whh_sb = [consts.tile([128, NJ, NK, 128], bf16, name=f"whh{i}_sb")
                      for i in range(2)]
            nc.sync.dma_start(out=whh_sb[0], in_=whh0[:])
            nc.sync.dma_start(out=whh_sb[1], in_=whh1[:])
            gb_sb = [consts.tile([128, NJ], f32, name=f"gb{i}_sb") for i in range(2)]
            nc.sync.dma_start(out=gb_sb[0], in_=gb0[:])
            nc.sync.dma_start(out=gb_sb[1], in_=gb1[:])
            bhn_small = consts.tile([128, 2, NK], f32)
            nc.sync.dma_start(out=bhn_small, in_=bhnT[:])
            bhn_bc = consts.tile([128, 2, NK, bl], f32)
            nc.vector.tensor_copy(out=bhn_bc,
                                  in_=bhn_small.to_broadcast([128, 2, NK, bl]))
            id_sbb = consts.tile([128, 128], bf16)
            nc.sync.dma_start(out=id_sbb, in_=ident[:])
            lng_sb = consts.tile([128, NK], f32)
            nc.sync.dma_start(out=lng_sb, in_=lng[:])
            lnb2_sb = consts.tile([128, NK], f32)
            nc.sync.dma_start(out=lnb2_sb, in_=lnb2[:])
            wp_sb = consts.tile([128, NK, 256], f32r)
            nc.sync.dma_start(out=wp_sb, in_=wpT[:])
            bp_sb = consts.tile([128, 2], f32)
            nc.sync.dma_start(out=bp_sb, in_=bp[:])

            ones_stage = consts.tile([128, 128], f32)
            nc.vector.memset(ones_stage, 1.0)
            ones_col = consts.tile([128, 1], f32r)     # lhsT for partition-sum
            nc.vector.tensor_copy(out=ones_col, in_=ones_stage[:, 0:1])
            ones_row = consts.tile([1, 128], f32r)     # lhsT for partition-bcast
            nc.vector.tensor_copy(out=ones_row, in_=ones_stage[0:1, :])
            ones_colb = consts.tile([128, 1], bf16)    # lhsT for bf16 partition-sum
            nc.vector.tensor_copy(out=ones_colb, in_=ones_stage[:, 0:1])
            eps_sb = consts.tile([1, 1], f32)
            nc.vector.memset(eps_sb, EPS)

            hz = consts.tile([128, NK, bl], bf16)      # h(0) = 0
            nc.vector.memset(hz, 0.0)

            # E accumulators
            s1_acc = consts.tile([128, NK, bl], f32)   # sum_t h2*rs
            nc.vector.memset(s1_acc, 0.0)
            s2_acc = consts.tile([1, bl], f32)         # sum_t mu*rs
            nc.vector.memset(s2_acc, 0.0)
            rs_last = consts.tile([1, bl], f32r)
            mu_last = consts.tile([1, bl], f32r)

            tc.strict_bb_all_engine_barrier()

            with tc.tile_pool(name="gx0", bufs=2) as gx0p, \
                 tc.tile_pool(name="gx1", bufs=2) as gx1p, \
                 tc.tile_pool(name="h1", bufs=2) as h1p, \
                 tc.tile_pool(name="h2", bufs=2) as h2p, \
                 tc.tile_pool(name="xm", bufs=2) as xmp, \
                 tc.tile_pool(name="tmp", bufs=2) as tmp, \
                 tc.tile_pool(name="et", bufs=1) as etp, \
                 tc.tile_pool(name="ep", bufs=1) as ep:

                h2_keep = ep.tile([128, NK, bl], f32)  # h2(T) copy for epilogue

                with tc.tile_pool(name="ps0", bufs=2, space="PSUM") as ps0p, \
                     tc.tile_pool(name="ps1", bufs=2, space="PSUM") as ps1p, \
                     tc.tile_pool(name="psA", bufs=1, space="PSUM") as psAp, \
                     tc.tile_pool(name="psC", bufs=1, space="PSUM") as psCp, \
                     tc.tile_pool(name="psE", bufs=1, space="PSUM") as psEp:

                    # ---------------- emission helpers ----------------
                    ps_pool = [ps0p, ps1p]

                    def emit_scan_mms(l, gx_slot, i, h_prev):
                        """One scan step's matmul burst for layer l.
                        ps[:,j,:] = W_hh[j]@h (+ gx for r,z; + b_hh_n for n).
                        h_prev = (tile, islice or None)."""
                        ps = ps_pool[l].tile([128, NJ, bl], f32, tag=f"ps{l}")
                        tsl = ts(i, bl)
                        ht, hi = h_prev
                        def hk(k):
                            return ht[:, k, :] if hi is None \
                                else ht[:, k, ts(hi, bl)]
                        # n-gate groups first (j = 2NK..3NK)
                        for q in range(NK):
                            j = 2 * NK + q
                            for k in range(NK):
                                nc.tensor.matmul(ps[:, j, :], whh_sb[l][:, j, k, :],
                                                 hk(k),
                                                 start=(k == 0), stop=(k == NK - 1))
                        # r,z groups: gx identity-fold, then W
                        for j in range(2 * NK):
                            if ID_FOLD:
                                nc.tensor.matmul(ps[:, j, :], id_sbb,
                                                 gx_slot[:, j, tsl],
                                                 start=True, stop=False)
                            for k in range(NK):
                                nc.tensor.matmul(ps[:, j, :], whh_sb[l][:, j, k, :],
                                                 hk(k),
                                                 start=(k == 0 and not ID_FOLD),
                                                 stop=(k == NK - 1))
                        return ps

                    def emit_chain(l, ps, gx_slot, i, h_prev, h_out):
                        """Gate math for one step; writes bf16 h' into h_out."""
                        tsl = ts(i, bl)
                        ht, hi = h_prev
                        h_read = ht if hi is None else ht[:, :, ts(hi, bl)]
                        if ID_FOLD:
                            rz = tmp.tile([128, 2 * NK, bl], f32, tag=f"rz{l}")
                            nc.scalar.activation(out=rz, in_=ps[:, 0:2 * NK, :],
                                                 func=AF.Sigmoid)
                            u = tmp.tile([128, NK, bl], f32, tag=f"u{l}")
                            nc.scalar.activation(out=u, in_=ps[:, NK:2 * NK, :],
                                                 func=AF.Sigmoid, scale=-1.0)
                        else:
                            rzp = tmp.tile([128, 2 * NK, bl], f32, tag=f"rzp{l}")
                            nc.vector.tensor_add(rzp, ps[:, 0:2 * NK, :],
                                                 gx_slot[:, 0:2 * NK, tsl])
                            rz = tmp.tile([128, 2 * NK, bl], f32, tag=f"rz{l}")
                            nc.scalar.activation(out=rz, in_=rzp, func=AF.Sigmoid)
                            u = tmp.tile([128, NK, bl], f32, tag=f"u{l}")
                            nc.scalar.activation(out=u, in_=rzp[:, NK:2 * NK, :],
                                                 func=AF.Sigmoid, scale=-1.0)
                        t1 = tmp.tile([128, NK, bl], f32, tag=f"t1{l}")
                        nc.vector.tensor_mul(t1, rz[:, NK:2 * NK, :], h_read)
                        nb = tmp.tile([128, NK, bl], f32, tag=f"nb{l}")
                        nc.vector.tensor_add(nb, ps[:, 2 * NK:3 * NK, :],
                                             bhn_bc[:, l, :, :])
                        nh = tmp.tile([128, NK, bl], f32, tag=f"nh{l}")
                        nc.vector.tensor_mul(nh, nb, rz[:, 0:NK, :])
                        npre = tmp.tile([128, NK, bl], f32, tag=f"np{l}")
                        nc.vector.tensor_add(npre, nh,
                                             gx_slot[:, 2 * NK:3 * NK, tsl])
                        n = tmp.tile([128, NK, bl], f32, tag=f"n{l}")
                        nc.scalar.activation(out=n, in_=npre, func=AF.Tanh)
                        t2 = tmp.tile([128, NK, bl], f32, tag=f"t2{l}")
                        nc.vector.tensor_mul(t2, u, n)
                        nc.vector.tensor_add(h_out, t1, t2)

                    def emit_A_unit(j, xs, gx_slot):
                        ps = psAp.tile([128, ctok], f32, tag="A")
                        nc.tensor.matmul(ps, w0_sb[:, 0, ts(j, 128)],
                                         xs[:, 0, :], start=True, stop=False)
                        nc.tensor.matmul(ps, w0_sb[:, 1, ts(j, 128)],
                                         xs[:, 1, :], start=False, stop=True)
                        nc.scalar.activation(out=gx_slot[:, j, :], in_=ps,
                                             func=AF.Identity,
                                             bias=gb_sb[0][:, j:j + 1])

                    def emit_C_unit(j, h1_slot, gx_slot):
                        ps = psCp.tile([128, ctok], f32, tag="C")
                        for k in range(NK):
                            nc.tensor.matmul(ps, w1_sb[:, k, ts(j, 128)],
                                             h1_slot[:, k, :],
                                             start=(k == 0), stop=(k == NK - 1))
                        nc.vector.tensor_scalar_add(gx_slot[:, j, :], ps,
                                                    gb_sb[1][:, j:j + 1])

                    def emit_E(h2_slot, is_last):
                        """LN stats + pooling accumulation over one chunk."""
                        sq = etp.tile([128, NK, ctok], bf16, tag="sq")
                        nc.scalar.activation(out=sq, in_=h2_slot, func=AF.Square)
                        pss = psEp.tile([1, ctok], f32, tag="ps")
                        for k in range(NK):
                            nc.tensor.matmul(pss, ones_colb, h2_slot[:, k, :],
                                             start=(k == 0), stop=(k == NK - 1))
                        # packed per-token scratch: 0=mu 1=mu2/sd/mrs 2=var
                        esc = etp.tile([1, 4, ctok], f32, tag="esc")
                        mu = esc[:, 0, :]
                        nc.vector.tensor_scalar_mul(mu, pss, 1.0 / H)
                        psq = psEp.tile([1, ctok], f32, tag="ps")
                        for k in range(NK):
                            nc.tensor.matmul(psq, ones_colb, sq[:, k, :],
                                             start=(k == 0), stop=(k == NK - 1))
                        mu2 = esc[:, 1, :]
                        nc.vector.tensor_mul(mu2, mu, mu)
                        var = esc[:, 2, :]
                        nc.vector.scalar_tensor_tensor(var, psq, 1.0 / H, mu2,
                                                       op0=AluOpType.mult,
                                                       op1=AluOpType.subtract)
                        sd = esc[:, 1, :]
                        nc.scalar.activation(out=sd, in_=var, func=AF.Sqrt,
                                             bias=eps_sb)
                        rs = etp.tile([1, ctok], f32r, tag="rs")
                        with nc.allow_low_precision(reason="f32r is full-width fp32"):
                            nc.vector.reciprocal(rs, sd)
                        brs = psEp.tile([128, ctok], f32, tag="brs")
                        nc.tensor.matmul(brs, ones_row, rs)
                        for k in range(NK):
                            t = etp.tile([128, ctok], f32, tag="et")
                            nc.vector.tensor_mul(t, h2_slot[:, k, :], brs)
                            red = etp.tile([128, bl], f32, tag="red")
                            nc.vector.tensor_reduce(
                                red, t.rearrange("p (t b) -> p b t", b=bl),
                                axis=AX.X, op=AluOpType.add)
                            nc.vector.tensor_add(s1_acc[:, k, :], s1_acc[:, k, :],
                                                 red)
                        mrs = esc[:, 1, :]
                        nc.vector.tensor_mul(mrs, mu, rs.bitcast(f32))
                        redm = etp.tile([1, bl], f32, tag="redm")
                        nc.vector.tensor_reduce(
                            redm, mrs.rearrange("p (t b) -> p b t", b=bl),
                            axis=AX.X, op=AluOpType.add)
                        nc.vector.tensor_add(s2_acc, s2_acc, redm)
                        if is_last:
                            nc.vector.tensor_copy(out=rs_last,
                                                  in_=rs[:, (CH - 1) * bl:].bitcast(f32))
                            nc.vector.tensor_copy(out=mu_last,
                                                  in_=mu[:, (CH - 1) * bl:])
                            nc.vector.tensor_copy(
                                out=h2_keep, in_=h2_slot[:, :, (CH - 1) * bl:])

                    # ---------------- main pipeline ----------------
                    # iteration c: A(c+1) fillers, L0 chunk c, C(c-1) fillers,
                    #              L1 chunk c-2, E(c-3)
                    L1_LAG = 2
                    xm_tiles = {}
                    gx0_slot = {}
                    gx1_slot = {}
                    h1_slot = {}
                    h2_slot = {}
                    h1_prev = (hz, None)  # h-state entering next L0 step
                    h2_prev = (hz, None)

                    def load_xm(c):
                        xs = xmp.tile([F, 2, ctok], bf16, tag="xm", name="xms")
                        nc.sync.dma_start(
                            out=xs,
                            in_=xmT[:, ds(c * ctok, ctok)].rearrange(
                                "(k f) t -> f k t", k=2))
                        xm_tiles[c] = xs

                    # prologue: A(0) fully, so L0 can start immediately
                    load_xm(0)
                    gx0_slot[0] = gx0p.tile([128, NJ, ctok], bf16, tag="gx0", name="gx0s")
                    for j in range(NJ):
                        emit_A_unit(j, xm_tiles[0], gx0_slot[0])
                    del xm_tiles[0]
                    load_xm(1)

                    for c in range(nch + L1_LAG + 1):
                        a_c = c + 1          # A chunk this iteration
                        l0_c = c             # L0 chunk
                        c_c = c - 1          # C chunk
                        l1_c = c - L1_LAG    # L1 chunk
                        e_c = c - L1_LAG - 1 # E chunk

                        if a_c < nch:
                            gx0_slot[a_c] = gx0p.tile([128, NJ, ctok], bf16,
                                                      tag="gx0", name="gx0s")
                            if a_c + 1 < nch:
                                load_xm(a_c + 1)
                        if l0_c < nch:
                            h1_slot[l0_c] = h1p.tile([128, NK, ctok], bf16,
                                                     tag="h1", name="h1s")
                        if 0 <= c_c < nch:
                            gx1_slot[c_c] = gx1p.tile([128, NJ, ctok], bf16,
                                                      tag="gx1", name="gx1s")
                        if 0 <= l1_c < nch:
                            h2_slot[l1_c] = h2p.tile([128, NK, ctok], bf16,
                                                     tag="h2", name="h2s")

                        for i in range(CH):
                            if l0_c < nch:
                                ps0 = emit_scan_mms(0, gx0_slot[l0_c], i, h1_prev)
                            if 0 <= l1_c < nch:
                                ps1 = emit_scan_mms(1, gx1_slot[l1_c], i, h2_prev)
                            if l0_c < nch:
                                h_out = h1_slot[l0_c][:, :, ts(i, bl)]
                                emit_chain(0, ps0, gx0_slot[l0_c], i, h1_prev,
                                           h_out)
                                h1_prev = (h1_slot[l0_c], i)
                            if 0 <= l1_c < nch:
                                h_out = h2_slot[l1_c][:, :, ts(i, bl)]
                                emit_chain(1, ps1, gx1_slot[l1_c], i, h2_prev,
                                           h_out)
                                h2_prev = (h2_slot[l1_c], i)
                            # PE fillers
                            if i < NJ:
                                if a_c < nch:
                                    emit_A_unit(i, xm_tiles[a_c], gx0_slot[a_c])
                                if 0 <= c_c < nch:
                                    emit_C_unit(i, h1_slot[c_c], gx1_slot[c_c])

                        if a_c < nch:
                            del xm_tiles[a_c]
                        if 0 <= e_c < nch:
                            emit_E(h2_slot[e_c], is_last=(e_c == nch - 1))

                # ------------ epilogue: pool + proj + GELU ------------
                with tc.tile_pool(name="psF", bufs=1, space="PSUM") as psFp:
                    # broadcasts of per-token scalars to 128 partitions
                    bc = psFp.tile([128, 3, bl], f32, tag="bc")
                    s2t = ep.tile([1, bl], f32r)
                    nc.vector.tensor_scalar_mul(s2t, s2_acc, 1.0 / scan_T)
                    nc.tensor.matmul(bc[:, 0, :], ones_row, s2t)
                    nc.tensor.matmul(bc[:, 1, :], ones_row, mu_last)
                    nc.tensor.matmul(bc[:, 2, :], ones_row, rs_last)
                    # mean part: pm = S1/T - bcast(s2/T)
                    pm = ep.tile([128, NK, bl], f32)
                    nc.vector.scalar_tensor_tensor(
                        pm, s1_acc, 1.0 / scan_T,
                        bc[:, 0:1, :].to_broadcast([128, NK, bl]),
                        op0=AluOpType.mult, op1=AluOpType.subtract)
                    # last part: (h2_last - mu)*rs
                    hl = ep.tile([128, NK, bl], f32)
                    nc.vector.tensor_sub(
                        hl, h2_keep, bc[:, 1:2, :].to_broadcast([128, NK, bl]))
                    hlr = ep.tile([128, NK, bl], f32)
                    nc.vector.tensor_mul(
                        hlr, hl, bc[:, 2:3, :].to_broadcast([128, NK, bl]))
                    both = ep.tile([128, NK, bl], f32)
                    nc.vector.tensor_add(both, pm, hlr)
                    # pooled = g*both + 2*b  (LN affine applied to both terms)
                    po = ep.tile([128, NK, bl], f32r)
                    for k in range(NK):
                        nc.vector.tensor_scalar(po[:, k, :], both[:, k, :],
                                                lng_sb[:, k:k + 1],
                                                lnb2_sb[:, k:k + 1],
                                                op0=AluOpType.mult,
                                                op1=AluOpType.add)
                    for j in range(2):
                        psy = psFp.tile([128, bl], f32, tag="psy")
                        for k in range(NK):
                            nc.tensor.matmul(psy, r32(wp_sb[:, k, ts(j, 128)]),
                                             po[:, k, :],
                                             start=(k == 0), stop=(k == NK - 1))
                        yj = ep.tile([128, bl], f32, name=f"yj{j}")
                        nc.scalar.activation(
                            out=yj, in_=psy,
                            func=AF.Identity if SIM_MODE else AF.Gelu,
                            bias=bp_sb[:, j:j + 1])
                        nc.sync.dma_start(out=out[j], in_=yj)
    nc.finalize()
    return nc


# ---------------- host-side input prep ----------------

def prep_shared(W_ih0, W_hh0, b_ih0, b_hh0, W_ih1, W_hh1, b_ih1, b_hh1,
                ln_g, ln_b, W_proj, b_proj):
    def whh_tiles(W_hh):
        # [p, j, k, m] = W_hh^T[128k+p, 128j+m]
        w = np.ascontiguousarray(W_hh.T).reshape(NK, 128, NJ, 128)
        return np.ascontiguousarray(w.transpose(1, 2, 0, 3)).astype(ml_dtypes.bfloat16)

    def fold_bias(b_ih, b_hh):
        g = b_ih.copy()
        g[:2 * H] += b_hh[:2 * H]
        return np.ascontiguousarray(g.reshape(NJ, 128).T)  # [128, NJ]

    shared = {}
    w0 = np.ascontiguousarray(W_ih0.T)            # [130, 1536]
    shared["w0T"] = np.ascontiguousarray(w0.reshape(2, F, H3).transpose(1, 0, 2))
    w1 = np.ascontiguousarray(W_ih1.T)            # [512, 1536]
    shared["w1T"] = np.ascontiguousarray(w1.reshape(NK, 128, H3).transpose(1, 0, 2)).astype(ml_dtypes.bfloat16)
    shared["whh0"] = whh_tiles(W_hh0)
    shared["whh1"] = whh_tiles(W_hh1)
    shared["gb0"] = fold_bias(b_ih0, b_hh0)
    shared["gb1"] = fold_bias(b_ih1, b_hh1)
    bhn = np.stack([b_hh0[2 * H:].reshape(NK, 128),
                    b_hh1[2 * H:].reshape(NK, 128)])   # [2, NK, 128]
    shared["bhnT"] = np.ascontiguousarray(bhn.transpose(2, 0, 1))  # [128, 2, NK]
    shared["ident"] = np.eye(128, dtype=np.float32)
    shared["lng"] = np.ascontiguousarray(ln_g.reshape(NK, 128).T)
    shared["lnb2"] = np.ascontiguousarray((2.0 * ln_b).reshape(NK, 128).T)
    shared["wpT"] = np.ascontiguousarray(
        W_proj.T.reshape(NK, 128, 256).transpose(1, 0, 2))
    shared["bp"] = np.ascontiguousarray(b_proj.reshape(2, 128).T)
    shared = {k: np.asarray(v, dtype=(ml_dtypes.bfloat16
                                      if k in ("whh0", "whh1", "ident", "w0T", "w1T")
                                      else np.float32))
              for k, v in shared.items()}
    return shared


def prep_xmT(x_core, mask_core, scan_T=T, bl=BL):
    # xmT[f, t*bl + b] = concat(x, mask)[b, t, f]
    xm = np.concatenate([x_core, mask_core.astype(np.float32)], axis=-1)  # [bl,T,2F]
    return np.ascontiguousarray(xm.transpose(2, 1, 0).reshape(2 * F, scan_T * bl)
                                ).astype(ml_dtypes.bfloat16)


_CACHE = {}


def _enable_trace_support():
    """Profiling-only shim (used by test.py, not the graded path)."""
    import sys
    import types
    import concourse.bass_utils as bu
    bu.upload_artifacts = lambda tmpdir: "local://" + tmpdir
    try:
        from antenv.axon_hooks import get_axon_ntff_profile_hook  # noqa: F401
        return
    except ImportError:
        pass
    from trn_agent_boot.trn_boot import _ntff_profile_via_ctypes
    hook = _ntff_profile_via_ctypes("/opt/axon/libaxon_pjrt.so")
    mod = types.ModuleType("antenv.axon_hooks")
    mod.get_axon_ntff_profile_hook = lambda: hook
    mod.set_axon_ntff_profile_hook = lambda h: None
    sys.modules["antenv.axon_hooks"] = mod


def kernel(x, mask, W_ih0, W_hh0, b_ih0, b_hh0, W_ih1, W_hh1, b_ih1, b_hh1,
           ln_g, ln_b, W_proj, b_proj):
    from concourse.bass_utils import run_bass_kernel_spmd

    if "nc" not in _CACHE:
        _CACHE["nc"] = build_nc()
    nc = _CACHE["nc"]

    x = np.asarray(x, np.float32)
    mask = np.asarray(mask)
    shared = prep_shared(np.asarray(W_ih0, np.float32), np.asarray(W_hh0, np.float32),
                         np.asarray(b_ih0, np.float32), np.asarray(b_hh0, np.float32),
                         np.asarray(W_ih1, np.float32), np.asarray(W_hh1, np.float32),
                         np.asarray(b_ih1, np.float32), np.asarray(b_hh1, np.float32),
                         np.asarray(ln_g, np.float32), np.asarray(ln_b, np.float32),
                         np.asarray(W_proj, np.float32), np.asarray(b_proj, np.float32))
    in_maps = []
    for c in range(NCORES):
        m = dict(shared)
        m["xmT"] = prep_xmT(x[c * BL:(c + 1) * BL], mask[c * BL:(c + 1) * BL])
        in_maps.append(m)

    trace = os.environ.get("KERNEL_TRACE", "0") == "1"
    kw = {}
    if trace:
        _enable_trace_support()
        kw["tmpdir"] = os.environ.get("KERNEL_TRACE_DIR") or None
    res = run_bass_kernel_spmd(nc, in_maps, list(range(NCORES)), trace=trace, **kw)
    _CACHE["exec_time_ns"] = res.exec_time_ns
    if res.instructions_and_trace is not None:
        _CACHE["trace_path"] = res.instructions_and_trace[1]
    outs = []
    for c in range(NCORES):
        y = res.results[c]["out"]          # [2, 128, BL]
        outs.append(y.reshape(256, BL).T)  # [BL, 256]
    return np.ascontiguousarray(np.concatenate(outs, axis=0), dtype=np.float32)


# revision 17
# speedup vs baseline: 3.0496x; 1.2114x over previous
"""Trainium2 Bass kernel for nn_DataONEEncoder (2-layer GRU + LN + pool + proj + GELU).

Fully-fused pipeline, data-parallel over batch (B=256 -> 32 per core, 8 cores).

All intermediates stay in SBUF (no DRAM round trips for gx/h):
  - A-GEMM  : gx0 = xm @ W_ih0^T + b   computed chunk-by-chunk (16 steps) into an
              SBUF ring, interleaved into the scan as PE filler work.
  - L0 scan : GRU layer 0, one step per iteration.
  - C-GEMM  : gx1 = h1 @ W_ih1^T + b   from the L0 h-ring, PE filler work,
              one chunk behind L0.
  - L1 scan : GRU layer 1, two chunks behind L0 (so C can spread out).
  - E       : LayerNorm stats + pooling accumulation per chunk, three chunks
              behind L0;  mean_t LN(h2) = g*(sum_t h2*rs - sum_t mu*rs)/T + b.

Per scan step the gate-input adds (gx_r, gx_z) and b_hh(n) are folded into the
PSUM accumulation with identity / rank-1 matmuls, so the vector chain is only
5 tensor_tensor ops:  t1 = z*h ; nh = r*ps_n ; npre = nh+gx_n ; t2 = (1-z)*n ;
h' = t1+t2  (with r, z, 1-z, tanh on the scalar engine).  h is stored bf16.

The two layers' scans interleave at step granularity: while the PE runs one
layer's matmul burst, the DVE/ACT run the other layer's gate chain, keeping
the PE warm (HAM) and all engines busy.
"""

import os
import numpy as np
import ml_dtypes

import concourse.bass as bass
from concourse import bacc
import concourse.mybir as mybir
import concourse.tile as tile
from concourse.alu_op_type import AluOpType
from concourse.bass import ts, ds

B, T, F, H = 256, 512, 65, 512
NCORES = 8
BL = B // NCORES          # 32 batch per core
H3 = 3 * H                # 1536
NJ = H3 // 128            # 12 gate j-tiles
NK = H // 128             # 4 hidden k-tiles
EPS = 1e-5
CH = 16                   # scan steps per chunk

f32 = mybir.dt.float32
f32r = mybir.dt.float32r
bf16 = mybir.dt.bfloat16
AF = mybir.ActivationFunctionType
AX = mybir.AxisListType

SIM_MODE = os.environ.get("KERNEL_SIM", "0") == "1"   # CoreSim lacks Gelu
ID_FOLD = os.environ.get("KERNEL_IDFOLD", "1") == "1"


def r32(ap):
    return ap.bitcast(f32r)


def build_nc(scan_T=T, bl=BL):
    assert scan_T % CH == 0
    nch = scan_T // CH
    ctok = CH * bl
    nc = bacc.Bacc()

    # ---- external inputs (host pre-laid-out, see kernel()) ----
    xmT = nc.declare_dram_parameter("xmT", [2 * F, scan_T * bl], bf16, isOutput=False)
    w0T = nc.declare_dram_parameter("w0T", [F, 2, H3], bf16, isOutput=False)
    w1T = nc.declare_dram_parameter("w1T", [128, NK, H3], bf16, isOutput=False)
    whh0 = nc.declare_dram_parameter("whh0", [128, NJ, NK, 128], bf16, isOutput=False)
    whh1 = nc.declare_dram_parameter("whh1", [128, NJ, NK, 128], bf16, isOutput=False)
    gb0 = nc.declare_dram_parameter("gb0", [128, NJ], f32, isOutput=False)
    gb1 = nc.declare_dram_parameter("gb1", [128, NJ], f32, isOutput=False)
    bhnT = nc.declare_dram_parameter("bhnT", [128, 2, NK], f32, isOutput=False)
    ident = nc.declare_dram_parameter("ident", [128, 128], bf16, isOutput=False)
    lng = nc.declare_dram_parameter("lng", [128, NK], f32, isOutput=False)
    lnb2 = nc.declare_dram_parameter("lnb2", [128, NK], f32, isOutput=False)
    wpT = nc.declare_dram_parameter("wpT", [128, NK, 256], f32r, isOutput=False)
    bp = nc.declare_dram_parameter("bp", [128, 2], f32, isOutput=False)
    out = nc.declare_dram_parameter("out", [2, 128, bl], f32, isOutput=True)

    with tile.TileContext(nc) as tc:
        with tc.tile_pool(name="consts", bufs=1) as consts:

            # ---- constants to SBUF ----
            w0_sb = consts.tile([F, 2, H3], bf16)
            nc.sync.dma_start(out=w0_sb, in_=w0T[:])
            w1_sb = consts.tile([128, NK, H3], bf16)
            nc.sync.dma_start(out=w1_sb, in_=w1T[:])
            whh_sb = [consts.tile([128, NJ, NK, 128], bf16, name=f"whh{i}_sb")
                      for i in range(2)]
            nc.sync.dma_start(out=whh_sb[0], in_=whh0[:])
            nc.sync.dma_start(out=whh_sb[1], in_=whh1[:])
            gb_sb = [consts.tile([128, NJ], f32, name=f"gb{i}_sb") for i in range(2)]
            nc.sync.dma_start(out=gb_sb[0], in_=gb0[:])
            nc.sync.dma_start(out=gb_sb[1], in_=gb1[:])
            bhn_small = consts.tile([128, 2, NK], f32)
            nc.sync.dma_start(out=bhn_small, in_=bhnT[:])
            bhn_bcb = consts.tile([128, 2, NK, bl], bf16)
            nc.vector.tensor_copy(out=bhn_bcb,
                                  in_=bhn_small.to_broadcast([128, 2, NK, bl]))
            id_sbb = consts.tile([128, 128], bf16)
            nc.sync.dma_start(out=id_sbb, in_=ident[:])
            lng_sb = consts.tile([128, NK], f32)
            nc.sync.dma_start(out=lng_sb, in_=lng[:])
            lnb2_sb = consts.tile([128, NK], f32)
            nc.sync.dma_start(out=lnb2_sb, in_=lnb2[:])
            wp_sb = consts.tile([128, NK, 256], f32r)
            nc.sync.dma_start(out=wp_sb, in_=wpT[:])
            bp_sb = consts.tile([128, 2], f32)
            nc.sync.dma_start(out=bp_sb, in_=bp[:])

            ones_stage = consts.tile([128, 128], f32)
            nc.vector.memset(ones_stage, 1.0)
            ones_col = consts.tile([128, 1], f32r)     # lhsT for partition-sum
            nc.vector.tensor_copy(out=ones_col, in_=ones_stage[:, 0:1])
            ones_row = consts.tile([1, 128], f32r)     # lhsT for partition-bcast
            nc.vector.tensor_copy(out=ones_row, in_=ones_stage[0:1, :])
            ones_colb = consts.tile([128, 1], bf16)    # lhsT for bf16 partition-sum
            nc.vector.tensor_copy(out=ones_colb, in_=ones_stage[:, 0:1])
            eps_sb = consts.tile([1, 1], f32)
            nc.vector.memset(eps_sb, EPS)

            hz = consts.tile([128, NK, bl], bf16)      # h(0) = 0
            nc.vector.memset(hz, 0.0)

            # E accumulators
            s1_acc = consts.tile([128, NK, bl], f32)   # sum_t h2*rs
            nc.vector.memset(s1_acc, 0.0)
            s2_acc = consts.tile([1, bl], f32)         # sum_t mu*rs
            nc.vector.memset(s2_acc, 0.0)
            rs_last = consts.tile([1, bl], f32r)
            mu_last = consts.tile([1, bl], f32r)

            tc.strict_bb_all_engine_barrier()

            with tc.tile_pool(name="gx0", bufs=2) as gx0p, \
                 tc.tile_pool(name="gx1", bufs=2) as gx1p, \
                 tc.tile_pool(name="h1", bufs=2) as h1p, \
                 tc.tile_pool(name="h2", bufs=2) as h2p, \
                 tc.tile_pool(name="xm", bufs=2) as xmp, \
                 tc.tile_pool(name="tmp", bufs=2) as tmp, \
                 tc.tile_pool(name="et", bufs=1) as etp, \
                 tc.tile_pool(name="ep", bufs=1) as ep:

                h2_keep = ep.tile([128, NK, bl], f32)  # h2(T) copy for epilogue

                with tc.tile_pool(name="psrz0", bufs=1, space="PSUM") as psrz0p, \
                     tc.tile_pool(name="psn0", bufs=1, space="PSUM") as psn0p, \
                     tc.tile_pool(name="psrz1", bufs=1, space="PSUM") as psrz1p, \
                     tc.tile_pool(name="psn1", bufs=1, space="PSUM") as psn1p, \
                     tc.tile_pool(name="psA", bufs=1, space="PSUM") as psAp, \
                     tc.tile_pool(name="psC", bufs=1, space="PSUM") as psCp, \
                     tc.tile_pool(name="psE", bufs=1, space="PSUM") as psEp:

                    # ---------------- emission helpers ----------------
                    psrz_pool = [psrz0p, psrz1p]
                    psn_pool = [psn0p, psn1p]

                    def emit_scan_mms(l, gx_slot, i, h_prev):
                        """One scan step's matmul burst for layer l.
                        rz bank: W_hh[j]@h + gx (id-fold);  n bank: W@h + b_hh_n.
                        h_prev = (tile, islice or None)."""
                        ps_rz = psrz_pool[l].tile([128, 2 * NK, bl], f32,
                                                  tag=f"psrz{l}")
                        ps_n = psn_pool[l].tile([128, NK, bl], f32, tag=f"psn{l}")
                        tsl = ts(i, bl)
                        ht, hi = h_prev
                        def hk(k):
                            return ht[:, k, :] if hi is None \
                                else ht[:, k, ts(hi, bl)]
                        # r,z groups first so the sigmoid can start early
                        for j in range(2 * NK):
                            if ID_FOLD:
                                nc.tensor.matmul(ps_rz[:, j, :], id_sbb,
                                                 gx_slot[:, j, tsl],
                                                 start=True, stop=False)
                            for k in range(NK):
                                nc.tensor.matmul(ps_rz[:, j, :],
                                                 whh_sb[l][:, j, k, :], hk(k),
                                                 start=(k == 0 and not ID_FOLD),
                                                 stop=(k == NK - 1))
                        # n-gate groups: b_hh(n) id-fold, then W
                        for q in range(NK):
                            j = 2 * NK + q
                            nc.tensor.matmul(ps_n[:, q, :], id_sbb,
                                             bhn_bcb[:, l, q, :],
                                             start=True, stop=False)
                            for k in range(NK):
                                nc.tensor.matmul(ps_n[:, q, :],
                                                 whh_sb[l][:, j, k, :], hk(k),
                                                 start=False, stop=(k == NK - 1))
                        return ps_rz, ps_n

                    def emit_chain(l, ps2, gx_slot, i, h_prev, h_out):
                        """Gate math for one step; writes bf16 h' into h_out."""
                        ps_rz, ps_n = ps2
                        tsl = ts(i, bl)
                        ht, hi = h_prev
                        h_read = ht if hi is None else ht[:, :, ts(hi, bl)]
                        rz = tmp.tile([128, 2 * NK, bl], bf16, tag=f"rz{l}")
                        nc.scalar.activation(out=rz, in_=ps_rz, func=AF.Sigmoid)
                        # critical path: nh -> npre -> tanh -> t2 -> h'
                        nh = tmp.tile([128, NK, bl], bf16, tag=f"nh{l}")
                        nc.vector.tensor_mul(nh, ps_n, rz[:, 0:NK, :])
                        npre = tmp.tile([128, NK, bl], bf16, tag=f"np{l}")
                        nc.vector.tensor_add(npre, nh,
                                             gx_slot[:, 2 * NK:3 * NK, tsl])
                        n = tmp.tile([128, NK, bl], bf16, tag=f"n{l}")
                        nc.scalar.activation(out=n, in_=npre, func=AF.Tanh)
                        # off-path (run during tanh): u = 1-z, t1 = z*h
                        u = tmp.tile([128, NK, bl], bf16, tag=f"u{l}")
                        nc.vector.tensor_scalar(u, rz[:, NK:2 * NK, :], -1.0, 1.0,
                                                op0=AluOpType.mult,
                                                op1=AluOpType.add)
                        t1 = tmp.tile([128, NK, bl], bf16, tag=f"t1{l}")
                        nc.vector.tensor_mul(t1, rz[:, NK:2 * NK, :], h_read)
                        t2 = tmp.tile([128, NK, bl], bf16, tag=f"t2{l}")
                        nc.vector.tensor_mul(t2, u, n)
                        nc.vector.tensor_add(h_out, t1, t2)

                    def emit_A_unit(j, xs, gx_slot):
                        ps = psAp.tile([128, ctok], f32, tag="A")
                        nc.tensor.matmul(ps, w0_sb[:, 0, ts(j, 128)],
                                         xs[:, 0, :], start=True, stop=False)
                        nc.tensor.matmul(ps, w0_sb[:, 1, ts(j, 128)],
                                         xs[:, 1, :], start=False, stop=True)
                        nc.scalar.activation(out=gx_slot[:, j, :], in_=ps,
                                             func=AF.Identity,
                                             bias=gb_sb[0][:, j:j + 1])

                    def emit_C_unit(j, h1_slot, gx_slot):
                        ps = psCp.tile([128, ctok], f32, tag="C")
                        for k in range(NK):
                            nc.tensor.matmul(ps, w1_sb[:, k, ts(j, 128)],
                                             h1_slot[:, k, :],
                                             start=(k == 0), stop=(k == NK - 1))
                        if j % 2 == 0:
                            nc.scalar.activation(out=gx_slot[:, j, :], in_=ps,
                                                 func=AF.Identity,
                                                 bias=gb_sb[1][:, j:j + 1])
                        else:
                            nc.vector.tensor_scalar_add(gx_slot[:, j, :], ps,
                                                        gb_sb[1][:, j:j + 1])

                    def emit_E(h2_slot, is_last):
                        """LN stats + pooling accumulation over one chunk."""
                        sq = etp.tile([128, NK, ctok], bf16, tag="sq")
                        nc.scalar.activation(out=sq, in_=h2_slot, func=AF.Square)
                        pss = psEp.tile([1, ctok], f32, tag="ps")
                        for k in range(NK):
                            nc.tensor.matmul(pss, ones_colb, h2_slot[:, k, :],
                                             start=(k == 0), stop=(k == NK - 1))
                        # packed per-token scratch: 0=mu 1=mu2/sd/mrs 2=var
                        esc = etp.tile([1, 4, ctok], f32, tag="esc")
                        mu = esc[:, 0, :]
                        nc.vector.tensor_scalar_mul(mu, pss, 1.0 / H)
                        psq = psEp.tile([1, ctok], f32, tag="ps")
                        for k in range(NK):
                            nc.tensor.matmul(psq, ones_colb, sq[:, k, :],
                                             start=(k == 0), stop=(k == NK - 1))
                        mu2 = esc[:, 1, :]
                        nc.vector.tensor_mul(mu2, mu, mu)
                        var = esc[:, 2, :]
                        nc.vector.scalar_tensor_tensor(var, psq, 1.0 / H, mu2,
                                                       op0=AluOpType.mult,
                                                       op1=AluOpType.subtract)
                        sd = esc[:, 1, :]
                        nc.scalar.activation(out=sd, in_=var, func=AF.Sqrt,
                                             bias=eps_sb)
                        rs = etp.tile([1, ctok], f32r, tag="rs")
                        with nc.allow_low_precision(reason="f32r is full-width fp32"):
                            nc.vector.reciprocal(rs, sd)
                        brs_ps = psEp.tile([128, ctok], f32, tag="brs")
                        nc.tensor.matmul(brs_ps, ones_row, rs)
                        brs = etp.tile([128, ctok], bf16, tag="brssb")
                        nc.scalar.activation(out=brs, in_=brs_ps, func=AF.Identity)
                        for k in range(NK):
                            t = etp.tile([128, ctok], bf16, tag="et")
                            nc.gpsimd.tensor_mul(t, h2_slot[:, k, :], brs)
                            red = etp.tile([128, bl], f32, tag="red")
                            nc.vector.tensor_reduce(
                                red, t.rearrange("p (t b) -> p b t", b=bl),
                                axis=AX.X, op=AluOpType.add)
                            nc.gpsimd.tensor_add(s1_acc[:, k, :], s1_acc[:, k, :],
                                                 red)
                        mrs = esc[:, 1, :]
                        nc.gpsimd.tensor_mul(mrs, mu, rs.bitcast(f32))
                        redm = etp.tile([1, bl], f32, tag="redm")
                        nc.vector.tensor_reduce(
                            redm, mrs.rearrange("p (t b) -> p b t", b=bl),
                            axis=AX.X, op=AluOpType.add)
                        nc.gpsimd.tensor_add(s2_acc, s2_acc, redm)
                        if is_last:
                            nc.vector.tensor_copy(out=rs_last,
                                                  in_=rs[:, (CH - 1) * bl:].bitcast(f32))
                            nc.vector.tensor_copy(out=mu_last,
                                                  in_=mu[:, (CH - 1) * bl:])
                            nc.vector.tensor_copy(
                                out=h2_keep, in_=h2_slot[:, :, (CH - 1) * bl:])

                    # ---------------- main pipeline ----------------
                    # iteration c: A(c+1) fillers, L0 chunk c, C(c-1) fillers,
                    #              L1 chunk c-2, E(c-3)
                    L1_LAG = 2
                    xm_tiles = {}
                    gx0_slot = {}
                    gx1_slot = {}
                    h1_slot = {}
                    h2_slot = {}
                    h1_prev = (hz, None)  # h-state entering next L0 step
                    h2_prev = (hz, None)

                    def load_xm(c):
                        xs = xmp.tile([F, 2, ctok], bf16, tag="xm", name="xms")
                        nc.sync.dma_start(
                            out=xs,
                            in_=xmT[:, ds(c * ctok, ctok)].rearrange(
                                "(k f) t -> f k t", k=2))
                        xm_tiles[c] = xs

                    # prologue: A(0) fully, so L0 can start immediately
                    load_xm(0)
                    gx0_slot[0] = gx0p.tile([128, NJ, ctok], bf16, tag="gx0", name="gx0s")
                    for j in range(NJ):
                        emit_A_unit(j, xm_tiles[0], gx0_slot[0])
                    del xm_tiles[0]
                    load_xm(1)

                    for c in range(nch + L1_LAG + 1):
                        a_c = c + 1          # A chunk this iteration
                        l0_c = c             # L0 chunk
                        c_c = c - 1          # C chunk
                        l1_c = c - L1_LAG    # L1 chunk
                        e_c = c - L1_LAG - 1 # E chunk

                        if a_c < nch:
                            gx0_slot[a_c] = gx0p.tile([128, NJ, ctok], bf16,
                                                      tag="gx0", name="gx0s")
                            if a_c + 1 < nch:
                                load_xm(a_c + 1)
                        if l0_c < nch:
                            h1_slot[l0_c] = h1p.tile([128, NK, ctok], bf16,
                                                     tag="h1", name="h1s")
                        if 0 <= c_c < nch:
                            gx1_slot[c_c] = gx1p.tile([128, NJ, ctok], bf16,
                                                      tag="gx1", name="gx1s")
                        if 0 <= l1_c < nch:
                            h2_slot[l1_c] = h2p.tile([128, NK, ctok], bf16,
                                                     tag="h2", name="h2s")

                        for i in range(CH):
                            if l0_c < nch:
                                ps0 = emit_scan_mms(0, gx0_slot[l0_c], i, h1_prev)
                            if 0 <= l1_c < nch:
                                ps1 = emit_scan_mms(1, gx1_slot[l1_c], i, h2_prev)
                            if l0_c < nch:
                                h_out = h1_slot[l0_c][:, :, ts(i, bl)]
                                emit_chain(0, ps0, gx0_slot[l0_c], i, h1_prev,
                                           h_out)
                                h1_prev = (h1_slot[l0_c], i)
                            if 0 <= l1_c < nch:
                                h_out = h2_slot[l1_c][:, :, ts(i, bl)]
                                emit_chain(1, ps1, gx1_slot[l1_c], i, h2_prev,
                                           h_out)
                                h2_prev = (h2_slot[l1_c], i)
                            # PE fillers
                            if i < NJ:
                                if a_c < nch:
                                    emit_A_unit(i, xm_tiles[a_c], gx0_slot[a_c])
                                if 0 <= c_c < nch:
                                    emit_C_unit(i, h1_slot[c_c], gx1_slot[c_c])

                        if a_c < nch:
                            del xm_tiles[a_c]
                        if 0 <= e_c < nch:
                            emit_E(h2_slot[e_c], is_last=(e_c == nch - 1))

                # ------------ epilogue: pool + proj + GELU ------------
                with tc.tile_pool(name="psF", bufs=1, space="PSUM") as psFp:
                    # broadcasts of per-token scalars to 128 partitions
                    bc = psFp.tile([128, 3, bl], f32, tag="bc")
                    s2t = ep.tile([1, bl], f32r)
                    nc.vector.tensor_scalar_mul(s2t, s2_acc, 1.0 / scan_T)
                    nc.tensor.matmul(bc[:, 0, :], ones_row, s2t)
                    nc.tensor.matmul(bc[:, 1, :], ones_row, mu_last)
                    nc.tensor.matmul(bc[:, 2, :], ones_row, rs_last)
                    # mean part: pm = S1/T - bcast(s2/T)
                    pm = ep.tile([128, NK, bl], f32)
                    nc.vector.scalar_tensor_tensor(
                        pm, s1_acc, 1.0 / scan_T,
                        bc[:, 0:1, :].to_broadcast([128, NK, bl]),
                        op0=AluOpType.mult, op1=AluOpType.subtract)
                    # last part: (h2_last - mu)*rs
                    hl = ep.tile([128, NK, bl], f32)
                    nc.vector.tensor_sub(
                        hl, h2_keep, bc[:, 1:2, :].to_broadcast([128, NK, bl]))
                    hlr = ep.tile([128, NK, bl], f32)
                    nc.vector.tensor_mul(
                        hlr, hl, bc[:, 2:3, :].to_broadcast([128, NK, bl]))
                    both = ep.tile([128, NK, bl], f32)
                    nc.vector.tensor_add(both, pm, hlr)
                    # pooled = g*both + 2*b  (LN affine applied to both terms)
                    po = ep.tile([128, NK, bl], f32r)
                    for k in range(NK):
                        nc.vector.tensor_scalar(po[:, k, :], both[:, k, :],
                                                lng_sb[:, k:k + 1],
                                                lnb2_sb[:, k:k + 1],
                                                op0=AluOpType.mult,
                                                op1=AluOpType.add)
                    for j in range(2):
                        psy = psFp.tile([128, bl], f32, tag="psy")
                        for k in range(NK):
                            nc.tensor.matmul(psy, r32(wp_sb[:, k, ts(j, 128)]),
                                             po[:, k, :],
                                             start=(k == 0), stop=(k == NK - 1))
                        yj = ep.tile([128, bl], f32, name=f"yj{j}")
                        nc.scalar.activation(
                            out=yj, in_=psy,
                            func=AF.Identity if SIM_MODE else AF.Gelu,
                            bias=bp_sb[:, j:j + 1])
                        nc.sync.dma_start(out=out[j], in_=yj)
    nc.finalize()
    return nc


# ---------------- host-side input prep ----------------

def prep_shared(W_ih0, W_hh0, b_ih0, b_hh0, W_ih1, W_hh1, b_ih1, b_hh1,
                ln_g, ln_b, W_proj, b_proj):
    def whh_tiles(W_hh):
        # [p, j, k, m] = W_hh^T[128k+p, 128j+m]
        w = np.ascontiguousarray(W_hh.T).reshape(NK, 128, NJ, 128)
        return np.ascontiguousarray(w.transpose(1, 2, 0, 3)).astype(ml_dtypes.bfloat16)

    def fold_bias(b_ih, b_hh):
        g = b_ih.copy()
        g[:2 * H] += b_hh[:2 * H]
        return np.ascontiguousarray(g.reshape(NJ, 128).T)  # [128, NJ]

    shared = {}
    w0 = np.ascontiguousarray(W_ih0.T)            # [130, 1536]
    shared["w0T"] = np.ascontiguousarray(w0.reshape(2, F, H3).transpose(1, 0, 2))
    w1 = np.ascontiguousarray(W_ih1.T)            # [512, 1536]
    shared["w1T"] = np.ascontiguousarray(w1.reshape(NK, 128, H3).transpose(1, 0, 2)).astype(ml_dtypes.bfloat16)
    shared["whh0"] = whh_tiles(W_hh0)
    shared["whh1"] = whh_tiles(W_hh1)
    shared["gb0"] = fold_bias(b_ih0, b_hh0)
    shared["gb1"] = fold_bias(b_ih1, b_hh1)
    bhn = np.stack([b_hh0[2 * H:].reshape(NK, 128),
                    b_hh1[2 * H:].reshape(NK, 128)])   # [2, NK, 128]
    shared["bhnT"] = np.ascontiguousarray(bhn.transpose(2, 0, 1))  # [128, 2, NK]
    shared["ident"] = np.eye(128, dtype=np.float32)
    shared["lng"] = np.ascontiguousarray(ln_g.reshape(NK, 128).T)
    shared["lnb2"] = np.ascontiguousarray((2.0 * ln_b).reshape(NK, 128).T)
    shared["wpT"] = np.ascontiguousarray(
        W_proj.T.reshape(NK, 128, 256).transpose(1, 0, 2))
    shared["bp"] = np.ascontiguousarray(b_proj.reshape(2, 128).T)
    shared = {k: np.asarray(v, dtype=(ml_dtypes.bfloat16
                                      if k in ("whh0", "whh1", "ident", "w0T", "w1T")
                                      else np.float32))
              for k, v in shared.items()}
    return shared


def prep_xmT(x_core, mask_core, scan_T=T, bl=BL):
    # xmT[f, t*bl + b] = concat(x, mask)[b, t, f]
    xm = np.concatenate([x_core, mask_core.astype(np.float32)], axis=-1)  # [bl,T,2F]
    return np.ascontiguousarray(xm.transpose(2, 1, 0).reshape(2 * F, scan_T * bl)
                                ).astype(ml_dtypes.bfloat16)


_CACHE = {}


def _enable_trace_support():
    """Profiling-only shim (used by test.py, not the graded path)."""
    import sys
    import types
    import concourse.bass_utils as bu
    bu.upload_artifacts = lambda tmpdir: "local://" + tmpdir
    try:
        from antenv.axon_hooks import get_axon_ntff_profile_hook  # noqa: F401
        return
    except ImportError:
        pass
    from trn_agent_boot.trn_boot import _ntff_profile_via_ctypes
    hook = _ntff_profile_via_ctypes("/opt/axon/libaxon_pjrt.so")
    mod = types.ModuleType("antenv.axon_hooks")
    mod.get_axon_ntff_profile_hook = lambda: hook
    mod.set_axon_ntff_profile_hook = lambda h: None
    sys.modules["antenv.axon_hooks"] = mod


def kernel(x, mask, W_ih0, W_hh0, b_ih0, b_hh0, W_ih1, W_hh1, b_ih1, b_hh1,
           ln_g, ln_b, W_proj, b_proj):
    from concourse.bass_utils import run_bass_kernel_spmd

    if "nc" not in _CACHE:
        _CACHE["nc"] = build_nc()
    nc = _CACHE["nc"]

    x = np.asarray(x, np.float32)
    mask = np.asarray(mask)
    shared = prep_shared(np.asarray(W_ih0, np.float32), np.asarray(W_hh0, np.float32),
                         np.asarray(b_ih0, np.float32), np.asarray(b_hh0, np.float32),
                         np.asarray(W_ih1, np.float32), np.asarray(W_hh1, np.float32),
                         np.asarray(b_ih1, np.float32), np.asarray(b_hh1, np.float32),
                         np.asarray(ln_g, np.float32), np.asarray(ln_b, np.float32),
                         np.asarray(W_proj, np.float32), np.asarray(b_proj, np.float32))
    in_maps = []
    for c in range(NCORES):
        m = dict(shared)
        m["xmT"] = prep_xmT(x[c * BL:(c + 1) * BL], mask[c * BL:(c + 1) * BL])
        in_maps.append(m)

    trace = os.environ.get("KERNEL_TRACE", "0") == "1"
    kw = {}
    if trace:
        _enable_trace_support()
        kw["tmpdir"] = os.environ.get("KERNEL_TRACE_DIR") or None
    res = run_bass_kernel_spmd(nc, in_maps, list(range(NCORES)), trace=trace, **kw)
    _CACHE["exec_time_ns"] = res.exec_time_ns
    if res.instructions_and_trace is not None:
        _CACHE["trace_path"] = res.instructions_and_trace[1]
    outs = []
    for c in range(NCORES):
        y = res.results[c]["out"]          # [2, 128, BL]
        outs.append(y.reshape(256, BL).T)  # [BL, 256]
    return np.ascontiguousarray(np.concatenate(outs, axis=0), dtype=np.float32)
